# revision 1
# baseline (speedup 1.0000x reference)
"""GQA (RoPE + causal softmax) Trainium2 Bass kernel, 8-core SPMD.

Sharding: DP over batch (2) x TP over KV groups (4 quarters of heads).
Core c handles batch c//4 and head quarter c%4 (8 q-heads, 2 kv-heads).
Each core computes a partial o_proj ([S, D]); host sums 4 partials per batch.

All matmuls run in float32r (TF32-like, 1 cyc/row at N>=256).
Everything on-chip is kept in "transposed" layout (feature dim on
partitions), which makes x^T the only host-side layout prep needed.
"""

import numpy as np

import concourse.bass as bass
import concourse.mybir as mybir
import concourse.tile as tile
from concourse import bacc, bass_utils

B, S, D = 2, 2048, 2048
H, KV, HD = 32, 8, 64
REP = H // KV
SCALE = 1.0 / 8.0  # 1/sqrt(HD)

F32 = mybir.dt.float32
F32R = mybir.dt.float32r
EXP = mybir.ActivationFunctionType.Exp

NCHUNK = S // 512        # 4 sq chunks of 512
NKT = D // 128           # 16 k-tiles over D
NST = S // 128           # 16 sk/s tiles

# local head j (0..7) -> denom row
def _pairrow(j):
    return 2 * (j % 4) + (j // 4)


def _build_program():
    nc = bacc.Bacc()

    xT = nc.dram_tensor("xT", [D, S], F32R, kind="ExternalInput").ap()
    wq = nc.dram_tensor("wq", [D, 8 * HD], F32R, kind="ExternalInput").ap()
    wk = nc.dram_tensor("wk", [D, 2 * HD], F32R, kind="ExternalInput").ap()
    wv = nc.dram_tensor("wv", [D, 2 * HD], F32R, kind="ExternalInput").ap()
    wo = nc.dram_tensor("wo", [8 * HD, D], F32R, kind="ExternalInput").ap()
    cosT2 = nc.dram_tensor("cosT2", [128, S], F32, kind="ExternalInput").ap()
    sinT2m = nc.dram_tensor("sinT2m", [128, S], F32, kind="ExternalInput").ap()
    tri = nc.dram_tensor("tri", [128, 128], F32, kind="ExternalInput").ap()
    ident = nc.dram_tensor("ident", [128, 64], F32R, kind="ExternalInput").ap()
    selA = nc.dram_tensor("selA", [128, 512], F32R, kind="ExternalInput").ap()
    selB = nc.dram_tensor("selB", [128, 512], F32R, kind="ExternalInput").ap()
    onescol = nc.dram_tensor("onescol", [128, 1], F32R, kind="ExternalInput").ap()
    zblk = nc.dram_tensor("zblk", [128, 128], F32R, kind="ExternalInput").ap()
    opart = nc.dram_tensor("opart", [S, D], F32, kind="ExternalOutput").ap()

    with tile.TileContext(nc) as tc:
        with (
            tc.tile_pool(name="persist", bufs=1) as pp,
            tc.tile_pool(name="consts", bufs=1) as cp,
        ):
            # persistent SBUF: q^T/k^T, attention outputs, small constants
            qT = [pp.tile([128, S], F32R, tag=f"qT{t}", name=f"qT{t}") for t in range(4)]
            kT = pp.tile([128, S], F32R, tag="kT")
            outT = [pp.tile([128, S], F32R, tag=f"outT{t}", name=f"outT{t}") for t in range(4)]
            denomA = pp.tile([128, S], F32, tag="denomA")
            denomB = pp.tile([128, S], F32, tag="denomB")
            trib = cp.tile([128, 128], F32, tag="trib")
            identb = cp.tile([128, 64], F32R, tag="identb")
            selAb = cp.tile([128, 512], F32R, tag="selAb")
            selBb = cp.tile([128, 512], F32R, tag="selBb")
            onesb = cp.tile([128, 1], F32R, tag="onesb")
            zblkb = cp.tile([128, 128], F32R, tag="zblkb")
            nc.sync.dma_start(trib[:], tri[:])
            nc.sync.dma_start(identb[:], ident[:])
            nc.sync.dma_start(selAb[:], selA[:])
            nc.sync.dma_start(selBb[:], selB[:])
            nc.sync.dma_start(onesb[:], onescol[:])
            nc.sync.dma_start(zblkb[:], zblk[:])
            nc.gpsimd.memset(denomA[:], 1.0)
            nc.gpsimd.memset(denomB[:], 1.0)

            vo = [[None] * NST, [None] * NST]
            with tc.tile_pool(name="vop", bufs=1) as vp:  # spans phases A..D
                with (
                    tc.tile_pool(name="ropec", bufs=1) as rcc,
                    tc.tile_pool(name="vtbuf", bufs=1) as vtb,
                ):
                    cosb = rcc.tile([128, S], F32, tag="cosb")
                    sinb = rcc.tile([128, S], F32, tag="sinb")
                    nc.sync.dma_start(cosb[:], cosT2[:])
                    nc.sync.dma_start(sinb[:], sinT2m[:])
                    vT = vtb.tile([128, S], F32R, tag="vT")

                    # ---------- Phase A: qkv^T = W^T @ x^T ----------
                    with (
                        tc.tile_pool(name="wts", bufs=1) as wp,
                        tc.tile_pool(name="xin", bufs=4) as xp,
                        tc.tile_pool(name="qkvps", bufs=6, space="PSUM") as pqkv,
                    ):
                        wqk = [wp.tile([128, 8 * HD], F32R, tag=f"wq{k}", name=f"wqk{k}") for k in range(NKT)]
                        wkk = [wp.tile([128, 2 * HD], F32R, tag=f"wk{k}", name=f"wkk{k}") for k in range(NKT)]
                        wvk = [wp.tile([128, 2 * HD], F32R, tag=f"wv{k}", name=f"wvk{k}") for k in range(NKT)]
                        for k in range(NKT):
                            r = slice(k * 128, (k + 1) * 128)
                            nc.sync.dma_start(wqk[k][:], wq[r, :])
                            nc.sync.dma_start(wkk[k][:], wk[r, :])
                            nc.sync.dma_start(wvk[k][:], wv[r, :])
                        for n in range(NCHUNK):
                            ncol = slice(n * 512, (n + 1) * 512)
                            accs = [pqkv.tile([128, 512], F32, tag="qkvacc", name=f"acc{n}_{m}") for m in range(6)]
                            for k in range(NKT):
                                xk = xp.tile([128, 512], F32R, tag="xk")
                                nc.sync.dma_start(xk[:], xT[k * 128:(k + 1) * 128, ncol])
                                st = k == 0
                                sp = k == NKT - 1
                                for t in range(4):
                                    nc.tensor.matmul(
                                        accs[t][:], wqk[k][:, t * 128:(t + 1) * 128],
                                        xk[:], start=st, stop=sp)
                                nc.tensor.matmul(accs[4][:], wkk[k][:], xk[:], start=st, stop=sp)
                                nc.tensor.matmul(accs[5][:], wvk[k][:], xk[:], start=st, stop=sp)
                            for t in range(4):
                                nc.vector.tensor_copy(qT[t][:, ncol], accs[t][:])
                            nc.vector.tensor_copy(kT[:, ncol], accs[4][:])
                            nc.vector.tensor_copy(vT[:, ncol], accs[5][:])

                    # ---------- Phase B: RoPE on q^T and k^T ----------
                    with tc.tile_pool(name="rope", bufs=2) as rp:
                        for tl in [*qT, kT]:
                            rot = rp.tile([128, S], F32, tag="rot")
                            tmp = rp.tile([128, S], F32, tag="tmp")
                            # rotate-half as partition-shifted copies (sign folded in sinb)
                            nc.gpsimd.tensor_copy(rot[0:32, :], tl[32:64, :])
                            nc.gpsimd.tensor_copy(rot[32:64, :], tl[0:32, :])
                            nc.gpsimd.tensor_copy(rot[64:96, :], tl[96:128, :])
                            nc.gpsimd.tensor_copy(rot[96:128, :], tl[64:96, :])
                            nc.vector.tensor_mul(tmp[:], tl[:], cosb[:])
                            nc.vector.tensor_mul(rot[:], rot[:], sinb[:])
                            nc.vector.tensor_add(tl[:], tmp[:], rot[:])

                    # ---------- Phase C: v natural tiles [128, 65] ----------
                    with tc.tile_pool(name="vtp", bufs=2, space="PSUM") as vtp:
                        for g in range(2):
                            for i in range(NST):
                                vps = vtp.tile([128, 64], F32R, tag="vps")
                                nc.tensor.transpose(
                                    vps[:], vT[g * 64:(g + 1) * 64, i * 128:(i + 1) * 128],
                                    identb[g * 64:(g + 1) * 64, :])
                                vt = vp.tile([128, 65], F32R, tag=f"vo{g}_{i}", name=f"vo{g}_{i}")
                                nc.vector.tensor_copy(vt[:, 0:64], vps[:])
                                nc.vector.tensor_copy(vt[:, 64:65], onesb[:])
                                vo[g][i] = vt

                # ---------- Phase D: attention ----------
                with (
                    tc.tile_pool(name="esb", bufs=10) as ep,
                    tc.tile_pool(name="sps", bufs=4, space="PSUM") as sp_,
                    tc.tile_pool(name="avp", bufs=3, space="PSUM") as ap_,
                ):
                    for t in range(4):
                        for j in range(NCHUNK):
                            jcol = slice(j * 512, (j + 1) * 512)
                            avs = []
                            for sub in range(2):
                                avs.append(ap_.tile([65, 512], F32, tag="avacc", name=f"av{t}_{j}_{sub}"))
                            for i in range(4 * j + 4):
                                c0 = max(0, 128 * (i - 4 * j))
                                ec0 = c0 if 512 - c0 >= 256 else 256
                                av0 = c0 if c0 < 384 else 256
                                for sub in range(2):
                                    pb = slice(64 * sub, 64 * sub + 64)
                                    g = sub
                                    ss = sp_.tile([128, 512], F32, tag="scps")
                                    nc.tensor.matmul(
                                        ss[:, ec0:512],
                                        kT[pb, i * 128:(i + 1) * 128],
                                        qT[t][pb, j * 512 + ec0:(j + 1) * 512],
                                        start=True, stop=True)
                                    es = ep.tile([128, 512], F32R, tag="es")
                                    nc.scalar.activation(
                                        es[:, c0:512], ss[:, c0:512], EXP, scale=SCALE)
                                    if i >= 4 * j:
                                        nc.vector.tensor_mul(
                                            es[:, c0:c0 + 128], es[:, c0:c0 + 128],
                                            trib[:])
                                    if c0 == 384:
                                        nc.vector.tensor_copy(es[:, 256:384], zblkb[:])
                                    nc.tensor.matmul(
                                        avs[sub][:, av0:512], vo[g][i][:],
                                        es[:, av0:512],
                                        start=(i == 0), stop=(i == 4 * j + 3))
                            for sub in range(2):
                                pb = slice(64 * sub, 64 * sub + 64)
                                nc.vector.tensor_copy(outT[t][pb, jcol], avs[sub][0:64, :])
                                dst = denomA if sub == 0 else denomB
                                nc.vector.tensor_copy(
                                    dst[32 * t:32 * t + 1, jcol], avs[sub][64:65, :])

            # ---------- Phase E: normalize + o_proj ----------
            with (
                tc.tile_pool(name="norm", bufs=2) as np_,
                tc.tile_pool(name="wop", bufs=1) as wop,
                tc.tile_pool(name="oout", bufs=3) as op,
                tc.tile_pool(name="bcps", bufs=2, space="PSUM") as bp_,
                tc.tile_pool(name="ops", bufs=4, space="PSUM") as opp,
                tc.tile_pool(name="rcp", bufs=1) as rcp,
            ):
                rcpf = rcp.tile([128, S], F32, tag="rcpf")
                rcprA = rcp.tile([128, S], F32R, tag="rcprA")
                rcprB = rcp.tile([128, S], F32R, tag="rcprB")
                for dt_, rr in ((denomA, rcprA), (denomB, rcprB)):
                    nc.vector.reciprocal(rcpf[:], dt_[:])
                    nc.vector.tensor_copy(rr[:], rcpf[:])
                for t in range(4):
                    tsl = slice(t * 128, (t + 1) * 128)
                    bcs = np_.tile([128, S], F32, tag="bcs")
                    for n in range(NCHUNK):
                        ncol = slice(n * 512, (n + 1) * 512)
                        bps = bp_.tile([128, 512], F32, tag="bps")
                        nc.tensor.matmul(
                            bps[:], selAb[:, tsl], rcprA[:, ncol],
                            start=True, stop=False)
                        nc.tensor.matmul(
                            bps[:], selBb[:, tsl], rcprB[:, ncol],
                            start=False, stop=True)
                        nc.vector.tensor_copy(bcs[:, ncol], bps[:])
                    nc.vector.tensor_mul(outT[t][:], outT[t][:], bcs[:])
                wot = [wop.tile([128, S], F32R, tag=f"wo{k}", name=f"wot{k}") for k in range(4)]
                for k in range(4):
                    nc.sync.dma_start(wot[k][:], wo[k * 128:(k + 1) * 128, :])
                for st in range(NST):
                    for dch in range(NCHUNK):
                        ops = opp.tile([128, 512], F32, tag="opps")
                        for kt in range(4):
                            nc.tensor.matmul(
                                ops[:], outT[kt][:, st * 128:(st + 1) * 128],
                                wot[kt][:, dch * 512:(dch + 1) * 512],
                                start=(kt == 0), stop=(kt == 3))
                        oo = op.tile([128, 512], F32, tag="oo")
                        nc.vector.tensor_copy(oo[:], ops[:])
                        nc.sync.dma_start(
                            opart[st * 128:(st + 1) * 128, dch * 512:(dch + 1) * 512],
                            oo[:])

    nc.compile()
    return nc


_PROGRAM = None


def _get_program():
    global _PROGRAM
    if _PROGRAM is None:
        _PROGRAM = _build_program()
    return _PROGRAM


def _make_in_maps(x, cos, sin, Wq, Wk, Wv, Wo):
    cosT = np.ascontiguousarray(cos.T.astype(np.float32))      # [64, S]
    sinT = np.ascontiguousarray(sin.T.astype(np.float32))
    cosT2 = np.tile(cosT, (2, 1))
    sinT2m = np.tile(np.concatenate([-sinT[:32], sinT[32:]], 0), (2, 1))
    tri = (np.arange(128)[None, :] >= np.arange(128)[:, None]).astype(np.float32)
    ident = np.tile(np.eye(64, dtype=np.float32), (2, 1))
    selA = np.zeros((128, 512), dtype=np.float32)
    selB = np.zeros((128, 512), dtype=np.float32)
    for t in range(4):
        selA[32 * t, 128 * t:128 * t + 64] = 1.0
        selB[32 * t, 128 * t + 64:128 * t + 128] = 1.0

    perm = [0, 4, 1, 5, 2, 6, 3, 7]
    in_maps = []
    for c in range(8):
        b, q = c // 4, c % 4
        idx = np.concatenate([np.arange(HD) + (8 * q + j) * HD for j in perm])
        in_maps.append({
            "xT": np.ascontiguousarray(x[b].T.astype(np.float32)),
            "wq": np.ascontiguousarray(Wq[:, idx].astype(np.float32)),
            "wk": np.ascontiguousarray(Wk[:, 2 * q * HD:(2 * q + 2) * HD].astype(np.float32)),
            "wv": np.ascontiguousarray(Wv[:, 2 * q * HD:(2 * q + 2) * HD].astype(np.float32)),
            "wo": np.ascontiguousarray(Wo[idx, :].astype(np.float32)),
            "cosT2": cosT2,
            "sinT2m": sinT2m,
            "tri": tri,
            "ident": ident,
            "selA": selA,
            "selB": selB,
            "onescol": np.ones((128, 1), dtype=np.float32),
            "zblk": np.zeros((128, 128), dtype=np.float32),
        })
    return in_maps


def _execute(in_maps, trace=False):
    nc = _get_program()
    return bass_utils.run_bass_kernel_spmd(
        nc, in_maps, core_ids=list(range(8)), trace=trace)


def kernel(x, cos, sin, Wq, Wk, Wv, Wo):
    in_maps = _make_in_maps(x, cos, sin, Wq, Wk, Wv, Wo)
    res = _execute(in_maps, trace=False)
    parts = [r["opart"] for r in res.results]
    out = np.empty((B, S, D), dtype=np.float32)
    for b in range(B):
        p = parts[4 * b:4 * b + 4]
        out[b] = (p[0] + p[1]) + (p[2] + p[3])
    return out



# revision 5
# speedup vs baseline: 1.1522x; 1.1522x over previous
"""GQA (RoPE + causal softmax) Trainium2 Bass kernel, 8-core SPMD.

Sharding: DP over batch (2) x TP over KV groups (4 quarters of heads).
Core c handles batch c//4 and head quarter c%4 (8 q-heads, 2 kv-heads).
Each core computes a partial o_proj ([S, D]); host sums 4 partials per batch.

All matmuls run in float32r (TF32-like, 1 cyc/col at N>=256, PE @2.4GHz).
Everything on-chip is kept in "transposed" layout (feature dim on
partitions), which makes x^T the only host-side layout prep needed.

Schedule (single fused pipeline, PE kept saturated):
 - Phase A: qkv^T = W^T @ x^T per 512-col chunk, with RoPE applied
   per chunk right after each chunk drains (rotate-half is a PE
   permutation matmul; the cos/sin multiplies+add run on DVE and
   overlap the next chunk's projection matmuls).
 - Phase C: v natural tiles [128, 65] via PE transposes (ones column
   for the softmax denominator via gpsimd memset).
 - Phase D: attention, j-outer (query-chunk outer). Scores are
   software-pipelined two blocks ahead of the exp->AV chain so the PE
   never stalls on the Scalar engine. After all heads finish a chunk,
   that chunk is normalized (reciprocal_approx_fast + select-matmul
   broadcast) and its o_proj rows run immediately, overlapping the
   next chunk's attention. attn outputs are written back into the qT
   tiles (their columns are dead once the chunk's scores are done).
"""

import numpy as np

import concourse.bass as bass
import concourse.mybir as mybir
import concourse.tile as tile
from concourse import bacc, bass_utils

B, S, D = 2, 2048, 2048
H, KV, HD = 32, 8, 64
REP = H // KV
SCALE = 1.0 / 8.0  # 1/sqrt(HD)

F32 = mybir.dt.float32
F32R = mybir.dt.float32r
EXP = mybir.ActivationFunctionType.Exp

NCHUNK = S // 512        # 4 sq chunks of 512
NKT = D // 128           # 16 k-tiles over D
NST = S // 128           # 16 sk/s tiles


def _build_program():
    nc = bacc.Bacc()

    xT = nc.dram_tensor("xT", [D, S], F32R, kind="ExternalInput").ap()
    wq = nc.dram_tensor("wq", [D, 8 * HD], F32R, kind="ExternalInput").ap()
    wk = nc.dram_tensor("wk", [D, 2 * HD], F32R, kind="ExternalInput").ap()
    wv = nc.dram_tensor("wv", [D, 2 * HD], F32R, kind="ExternalInput").ap()
    wo = nc.dram_tensor("wo", [8 * HD, D], F32R, kind="ExternalInput").ap()
    cosT2 = nc.dram_tensor("cosT2", [128, S], F32, kind="ExternalInput").ap()
    sinT2 = nc.dram_tensor("sinT2", [128, S], F32, kind="ExternalInput").ap()
    prot = nc.dram_tensor("prot", [128, 128], F32R, kind="ExternalInput").ap()
    tri = nc.dram_tensor("tri", [128, 128], F32, kind="ExternalInput").ap()
    ident = nc.dram_tensor("ident", [128, 64], F32R, kind="ExternalInput").ap()
    selA = nc.dram_tensor("selA", [128, 512], F32R, kind="ExternalInput").ap()
    selB = nc.dram_tensor("selB", [128, 512], F32R, kind="ExternalInput").ap()
    onescol = nc.dram_tensor("onescol", [128, 1], F32R, kind="ExternalInput").ap()
    zblk = nc.dram_tensor("zblk", [128, 128], F32R, kind="ExternalInput").ap()
    opart = nc.dram_tensor("opart", [S, D], F32, kind="ExternalOutput").ap()

    with tile.TileContext(nc) as tc:
        with (
            tc.tile_pool(name="persist", bufs=1) as pp,
            tc.tile_pool(name="consts", bufs=1) as cp,
        ):
            # persistent SBUF: q^T/k^T (qT doubles as attn-out storage),
            # denominators, small constants
            qT = [pp.tile([128, S], F32R, tag=f"qT{t}", name=f"qT{t}") for t in range(4)]
            kT = pp.tile([128, S], F32R, tag="kT")
            denomA = pp.tile([128, S], F32, tag="denomA")
            denomB = pp.tile([128, S], F32, tag="denomB")
            cosb = cp.tile([128, S], F32, tag="cosb")
            sinb = cp.tile([128, S], F32, tag="sinb")
            protb = cp.tile([128, 128], F32R, tag="protb")
            trib = cp.tile([128, 128], F32, tag="trib")
            identb = cp.tile([128, 64], F32R, tag="identb")
            selAb = cp.tile([128, 512], F32R, tag="selAb")
            selBb = cp.tile([128, 512], F32R, tag="selBb")
            onesb = cp.tile([128, 1], F32R, tag="onesb")
            zblkb = cp.tile([128, 128], F32R, tag="zblkb")
            wotp = pp.tile([128, 4 * D], F32R, tag="wotp")  # wo tiles, concat
            nc.gpsimd.memset(denomA[:], 1.0)
            nc.gpsimd.memset(denomB[:], 1.0)

            vo = [[None] * NST, [None] * NST]
            with tc.tile_pool(name="vop", bufs=1) as vp:  # spans C..D
                with tc.tile_pool(name="vtbuf", bufs=1) as vtb:
                    vT = vtb.tile([128, S], F32R, tag="vT")

                    # ---------- Phase A: qkv^T = W^T @ x^T, + RoPE ----------
                    with (
                        tc.tile_pool(name="wts", bufs=1) as wp,
                        tc.tile_pool(name="xin", bufs=4) as xp,
                        tc.tile_pool(name="qkvps", bufs=6, space="PSUM") as pqkv,
                        tc.tile_pool(name="rotps", bufs=2, space="PSUM") as rpp,
                        tc.tile_pool(name="ropet", bufs=2) as rtp,
                    ):
                        wqk = [wp.tile([128, 8 * HD], F32R, tag=f"wq{k}", name=f"wqk{k}") for k in range(NKT)]
                        wkk = [wp.tile([128, 2 * HD], F32R, tag=f"wk{k}", name=f"wkk{k}") for k in range(NKT)]
                        wvk = [wp.tile([128, 2 * HD], F32R, tag=f"wv{k}", name=f"wvk{k}") for k in range(NKT)]
                        # DMA order: k-tile weights first (first matmul can
                        # start ~2us in), consts woven between, wo last.
                        for k in range(NKT):
                            r = slice(k * 128, (k + 1) * 128)
                            nc.sync.dma_start(wqk[k][:], wq[r, :])
                            nc.sync.dma_start(wkk[k][:], wk[r, :])
                            nc.sync.dma_start(wvk[k][:], wv[r, :])
                            if k == 0:
                                nc.sync.dma_start(protb[:], prot[:])
                                nc.sync.dma_start(cosb[:], cosT2[:])
                                nc.sync.dma_start(sinb[:], sinT2[:])
                            if k == 2:
                                nc.sync.dma_start(trib[:], tri[:])
                                nc.sync.dma_start(identb[:], ident[:])
                                nc.sync.dma_start(selAb[:], selA[:])
                                nc.sync.dma_start(selBb[:], selB[:])
                                nc.sync.dma_start(onesb[:], onescol[:])
                                nc.sync.dma_start(zblkb[:], zblk[:])

                        for n in range(NCHUNK):
                            ncol = slice(n * 512, (n + 1) * 512)
                            accs = [pqkv.tile([128, 512], F32, tag="qkvacc", name=f"acc{n}_{m}") for m in range(6)]
                            for k in range(NKT):
                                xk = xp.tile([128, 512], F32R, tag="xk")
                                nc.sync.dma_start(xk[:], xT[k * 128:(k + 1) * 128, ncol])
                                st = k == 0
                                sp = k == NKT - 1
                                for t in range(4):
                                    nc.tensor.matmul(
                                        accs[t][:], wqk[k][:, t * 128:(t + 1) * 128],
                                        xk[:], start=st, stop=sp)
                                nc.tensor.matmul(accs[4][:], wkk[k][:], xk[:], start=st, stop=sp)
                                nc.tensor.matmul(accs[5][:], wvk[k][:], xk[:], start=st, stop=sp)
                            # drain + RoPE (q tiles and k); v drains plain
                            for m, tl in enumerate([*qT, kT]):
                                nc.vector.tensor_copy(tl[:, ncol], accs[m][:])
                                rps = rpp.tile([128, 512], F32, tag="rps")
                                nc.tensor.matmul(rps[:], protb[:], tl[:, ncol],
                                                 start=True, stop=True)
                                tmp = rtp.tile([128, 512], F32, tag="ropetmp")
                                nc.vector.tensor_mul(tmp[:], tl[:, ncol], cosb[:, ncol])
                                nc.vector.tensor_mul(rps[:], rps[:], sinb[:, ncol])
                                nc.vector.tensor_add(tl[:, ncol], tmp[:], rps[:])
                            nc.vector.tensor_copy(vT[:, ncol], accs[5][:])
                            if n == 0:
                                # wo DMA rides behind chunk-0 x tiles; needed
                                # only once o_proj starts (~130us in)
                                for k4 in range(4):
                                    nc.sync.dma_start(
                                        wotp[:, k4 * D:(k4 + 1) * D],
                                        wo[k4 * 128:(k4 + 1) * 128, :])

                    # ---------- Phase C: v natural tiles [128, 65] ----------
                    with tc.tile_pool(name="vtp", bufs=2, space="PSUM") as vtp:
                        for i in range(NST):
                            for g in range(2):
                                vps = vtp.tile([128, 64], F32R, tag="vps")
                                nc.tensor.transpose(
                                    vps[:], vT[g * 64:(g + 1) * 64, i * 128:(i + 1) * 128],
                                    identb[g * 64:(g + 1) * 64, :])
                                vt = vp.tile([128, 65], F32R, tag=f"vo{g}_{i}", name=f"vo{g}_{i}")
                                nc.vector.tensor_copy(vt[:, 0:64], vps[:])
                                nc.vector.tensor_copy(vt[:, 64:65], onesb[:])
                                vo[g][i] = vt

                # ---------- Phase D: attention + fused normalize/o_proj ----------
                with (
                    tc.tile_pool(name="esb", bufs=8) as ep,
                    tc.tile_pool(name="rcpp", bufs=2) as rcp_,
                    tc.tile_pool(name="oout", bufs=3) as op,
                    tc.tile_pool(name="sps", bufs=3, space="PSUM") as sp_,
                    tc.tile_pool(name="avp", bufs=2, space="PSUM") as ap_,
                    tc.tile_pool(name="bcps", bufs=1, space="PSUM") as bp_,
                    tc.tile_pool(name="ops", bufs=2, space="PSUM") as opp,
                ):
                    for j in range(NCHUNK):
                        jcol = slice(j * 512, (j + 1) * 512)
                        ni = 4 * j + 4
                        for t in range(4):
                            for sub in range(2):
                                pb = slice(64 * sub, 64 * sub + 64)
                                g = sub
                                avs = ap_.tile([65, 512], F32, tag="avacc",
                                               name=f"av{t}_{j}_{sub}")
                                ss = {}

                                def emit_score(i):
                                    c0 = max(0, 128 * (i - 4 * j))
                                    ec0 = c0 if 512 - c0 >= 256 else 256
                                    s = sp_.tile([128, 512], F32, tag="scps")
                                    nc.tensor.matmul(
                                        s[:, ec0:512],
                                        kT[pb, i * 128:(i + 1) * 128],
                                        qT[t][pb, j * 512 + ec0:(j + 1) * 512],
                                        start=True, stop=True)
                                    ss[i] = (s, c0)

                                def emit_exp_av(i):
                                    s, c0 = ss.pop(i)
                                    es = ep.tile([128, 512], F32R, tag="es")
                                    nc.scalar.activation(
                                        es[:, c0:512], s[:, c0:512], EXP, scale=SCALE)
                                    if i >= 4 * j:
                                        nc.vector.tensor_mul(
                                            es[:, c0:c0 + 128], es[:, c0:c0 + 128],
                                            trib[:])
                                    if c0 == 384:
                                        nc.vector.tensor_copy(es[:, 256:384], zblkb[:])
                                    av0 = c0 if c0 < 384 else 256
                                    nc.tensor.matmul(
                                        avs[:, av0:512], vo[g][i][:],
                                        es[:, av0:512],
                                        start=(i == 0), stop=(i == ni - 1))

                                emit_score(0)
                                if ni > 1:
                                    emit_score(1)
                                for i in range(ni):
                                    if i + 2 < ni:
                                        emit_score(i + 2)
                                    emit_exp_av(i)
                                # drain: attn out into qT (cols are dead), denom row
                                nc.vector.tensor_copy(qT[t][pb, jcol], avs[0:64, :])
                                dst = denomA if sub == 0 else denomB
                                nc.vector.tensor_copy(
                                    dst[32 * t:32 * t + 1, jcol], avs[64:65, :])

                        # normalize chunk j across all heads
                        nc.vector.reciprocal_approx_fast(
                            out=denomA[:, jcol], in_=denomA[:, jcol])
                        nc.vector.reciprocal_approx_fast(
                            out=denomB[:, jcol], in_=denomB[:, jcol])
                        rA = rcp_.tile([128, 512], F32R, tag="rA")
                        rB = rcp_.tile([128, 512], F32R, tag="rB")
                        nc.vector.tensor_copy(rA[:], denomA[:, jcol])
                        nc.vector.tensor_copy(rB[:], denomB[:, jcol])
                        rA = rA[:]
                        rB = rB[:]
                        for t in range(4):
                            tsl = slice(t * 128, (t + 1) * 128)
                            bps = bp_.tile([128, 512], F32, tag="bps")
                            nc.tensor.matmul(bps[:], selAb[:, tsl], rA,
                                             start=True, stop=False)
                            nc.tensor.matmul(bps[:], selBb[:, tsl], rB,
                                             start=False, stop=True)
                            nc.vector.tensor_mul(qT[t][:, jcol], qT[t][:, jcol], bps[:])

                        # o_proj rows for this chunk
                        for st in range(4 * j, 4 * j + 4):
                            stc = slice(st * 128, (st + 1) * 128)
                            for dch in range(NCHUNK):
                                ops = opp.tile([128, 512], F32, tag="opps")
                                for kt in range(4):
                                    nc.tensor.matmul(
                                        ops[:], qT[kt][:, stc],
                                        wotp[:, kt * D + dch * 512:kt * D + (dch + 1) * 512],
                                        start=(kt == 0), stop=(kt == 3))
                                oo = op.tile([128, 512], F32, tag="oo")
                                nc.vector.tensor_copy(oo[:], ops[:])
                                nc.sync.dma_start(
                                    opart[stc, dch * 512:(dch + 1) * 512],
                                    oo[:])

    nc.compile()
    return nc


_PROGRAM = None


def _get_program():
    global _PROGRAM
    if _PROGRAM is None:
        _PROGRAM = _build_program()
    return _PROGRAM


def _make_in_maps(x, cos, sin, Wq, Wk, Wv, Wo):
    cosT = np.ascontiguousarray(cos.T.astype(np.float32))      # [64, S]
    sinT = np.ascontiguousarray(sin.T.astype(np.float32))
    cosT2 = np.tile(cosT, (2, 1))
    sinT2 = np.tile(sinT, (2, 1))
    tri = (np.arange(128)[None, :] >= np.arange(128)[:, None]).astype(np.float32)
    ident = np.tile(np.eye(64, dtype=np.float32), (2, 1))
    # rotate-half permutation (sign folded): rot[m] = -tl[m+32] (m%64<32),
    # rot[m] = +tl[m-32] (m%64>=32); out[m,n] = sum_k prot[k,m]*tl[k,n]
    prot = np.zeros((128, 128), dtype=np.float32)
    for m in range(128):
        if m % 64 < 32:
            prot[m + 32, m] = -1.0
        else:
            prot[m - 32, m] = 1.0
    selA = np.zeros((128, 512), dtype=np.float32)
    selB = np.zeros((128, 512), dtype=np.float32)
    for t in range(4):
        selA[32 * t, 128 * t:128 * t + 64] = 1.0
        selB[32 * t, 128 * t + 64:128 * t + 128] = 1.0

    perm = [0, 4, 1, 5, 2, 6, 3, 7]
    in_maps = []
    for c in range(8):
        b, q = c // 4, c % 4
        idx = np.concatenate([np.arange(HD) + (8 * q + j) * HD for j in perm])
        in_maps.append({
            "xT": np.ascontiguousarray(x[b].T.astype(np.float32)),
            "wq": np.ascontiguousarray(Wq[:, idx].astype(np.float32)),
            "wk": np.ascontiguousarray(Wk[:, 2 * q * HD:(2 * q + 2) * HD].astype(np.float32)),
            "wv": np.ascontiguousarray(Wv[:, 2 * q * HD:(2 * q + 2) * HD].astype(np.float32)),
            "wo": np.ascontiguousarray(Wo[idx, :].astype(np.float32)),
            "cosT2": cosT2,
            "sinT2": sinT2,
            "prot": prot,
            "tri": tri,
            "ident": ident,
            "selA": selA,
            "selB": selB,
            "onescol": np.ones((128, 1), dtype=np.float32),
            "zblk": np.zeros((128, 128), dtype=np.float32),
        })
    return in_maps


def _execute(in_maps, trace=False):
    nc = _get_program()
    return bass_utils.run_bass_kernel_spmd(
        nc, in_maps, core_ids=list(range(8)), trace=trace)


def kernel(x, cos, sin, Wq, Wk, Wv, Wo):
    in_maps = _make_in_maps(x, cos, sin, Wq, Wk, Wv, Wo)
    res = _execute(in_maps, trace=False)
    parts = [r["opart"] for r in res.results]
    out = np.empty((B, S, D), dtype=np.float32)
    for b in range(B):
        p = parts[4 * b:4 * b + 4]
        out[b] = (p[0] + p[1]) + (p[2] + p[3])
    return out


# revision 7
# speedup vs baseline: 1.7232x; 1.4957x over previous
"""GQA (RoPE + causal softmax) Trainium2 Bass kernel, 8-core SPMD.

Sharding: DP over batch (2) x TP over KV groups (4 quarters of heads).
Core c handles batch c//4 and head quarter c%4 (8 q-heads, 2 kv-heads).
Each core computes a partial o_proj ([S, D]); host sums 4 partials per batch.

Matmuls run in float32r (1 col/cyc at N>=256, PE @2.4GHz); attention
probabilities and V are bf16. Everything on-chip is in "transposed"
layout (feature dim on partitions), so x^T is the only host-side prep.

Schedule (PE and Act kept concurrently busy):
 - Phase A: qkv^T = W^T @ x^T per 512-col chunk. Chunk-0 x DMAs are
   interleaved with the weight DMAs so the first matmul starts ~2us in.
   RoPE for chunk n (PE permutation matmul + DVE mul/mul/add) is
   emitted after chunk n+1's projection matmuls so it never stalls the
   PE. PSUM drains alternate DVE/Act.
 - Phase C: v natural tiles [128, 65] bf16 (ones column -> denominator
   rides the AV matmul).
 - Phase D (j-outer): scores for two key blocks go into one [128,1024]
   PSUM tile and get a single wide exp into a persistent bf16 SBUF
   tile. The score->exp chain is Act-paced; the PE's AV matmuls (one
   same-config block per head-pass, run one pass behind) and o_proj
   tiles (one chunk behind) are drawn from a backlog to fill the PE
   during Act waits. Chunk normalization uses reciprocal_approx_fast
   and a select-matmul partition broadcast; attention output reuses
   the qT tiles (their columns are dead after the chunk's scores).
"""

from collections import deque

import numpy as np

import concourse.bass as bass
import concourse.mybir as mybir
import concourse.tile as tile
from concourse import bacc, bass_utils

B, S, D = 2, 2048, 2048
H, KV, HD = 32, 8, 64
REP = H // KV
SCALE = 1.0 / 8.0  # 1/sqrt(HD)

F32 = mybir.dt.float32
F32R = mybir.dt.float32r
BF16 = mybir.dt.bfloat16
EXP = mybir.ActivationFunctionType.Exp

NCHUNK = S // 512        # 4 sq chunks of 512
NKT = D // 128           # 16 k-tiles over D
NST = S // 128           # 16 sk/s tiles


def _build_program():
    nc = bacc.Bacc()

    xT = nc.dram_tensor("xT", [D, S], F32R, kind="ExternalInput").ap()
    wq = nc.dram_tensor("wq", [D, 8 * HD], F32R, kind="ExternalInput").ap()
    wk = nc.dram_tensor("wk", [D, 2 * HD], F32R, kind="ExternalInput").ap()
    wv = nc.dram_tensor("wv", [D, 2 * HD], F32R, kind="ExternalInput").ap()
    wo = nc.dram_tensor("wo", [8 * HD, D], F32R, kind="ExternalInput").ap()
    cosT2 = nc.dram_tensor("cosT2", [128, S], F32, kind="ExternalInput").ap()
    sinT2 = nc.dram_tensor("sinT2", [128, S], F32, kind="ExternalInput").ap()
    prot = nc.dram_tensor("prot", [128, 128], F32R, kind="ExternalInput").ap()
    tri = nc.dram_tensor("tri", [128, 128], BF16, kind="ExternalInput").ap()
    ident = nc.dram_tensor("ident", [128, 64], F32R, kind="ExternalInput").ap()
    selA = nc.dram_tensor("selA", [128, 512], F32R, kind="ExternalInput").ap()
    selB = nc.dram_tensor("selB", [128, 512], F32R, kind="ExternalInput").ap()
    onescol = nc.dram_tensor("onescol", [128, 1], BF16, kind="ExternalInput").ap()
    zblk = nc.dram_tensor("zblk", [128, 128], BF16, kind="ExternalInput").ap()
    opart = nc.dram_tensor("opart", [S, D], F32, kind="ExternalOutput").ap()

    with tile.TileContext(nc) as tc:
        with (
            tc.tile_pool(name="persist", bufs=1) as pp,
            tc.tile_pool(name="consts", bufs=1) as cp,
        ):
            # persistent SBUF: q^T/k^T (qT doubles as attn-out storage),
            # denominators, small constants
            qT = [pp.tile([128, S], F32R, tag=f"qT{t}", name=f"qT{t}") for t in range(4)]
            kT = pp.tile([128, S], F32R, tag="kT")
            denomA = pp.tile([128, S], F32, tag="denomA")
            denomB = pp.tile([128, S], F32, tag="denomB")
            cosb = cp.tile([128, S], F32, tag="cosb")
            sinb = cp.tile([128, S], F32, tag="sinb")
            protb = cp.tile([128, 128], F32R, tag="protb")
            trib = cp.tile([128, 128], BF16, tag="trib")
            identb = cp.tile([128, 64], F32R, tag="identb")
            selAb = cp.tile([128, 512], F32R, tag="selAb")
            selBb = cp.tile([128, 512], F32R, tag="selBb")
            onesb = cp.tile([128, 1], BF16, tag="onesb")
            zblkb = cp.tile([128, 128], BF16, tag="zblkb")
            wotp = pp.tile([128, 4 * D], F32R, tag="wotp")  # wo tiles, concat
            nc.gpsimd.memset(denomA[:], 1.0)
            nc.gpsimd.memset(denomB[:], 1.0)

            vo = [[None] * NST, [None] * NST]
            with tc.tile_pool(name="vop", bufs=1) as vp:  # spans C..D
                with tc.tile_pool(name="vtbuf", bufs=1) as vtb:
                    vT = vtb.tile([128, S], F32R, tag="vT")

                    # ---------- Phase A: qkv^T = W^T @ x^T, + RoPE ----------
                    with (
                        tc.tile_pool(name="wts", bufs=1) as wp,
                        tc.tile_pool(name="xin", bufs=12) as xp,
                        tc.tile_pool(name="qkvps", bufs=6, space="PSUM") as pqkv,
                        tc.tile_pool(name="rotps", bufs=2, space="PSUM") as rpp,
                        tc.tile_pool(name="ropet", bufs=2) as rtp,
                    ):
                        wqk = [wp.tile([128, 8 * HD], F32R, tag=f"wq{k}", name=f"wqk{k}") for k in range(NKT)]
                        wkk = [wp.tile([128, 2 * HD], F32R, tag=f"wk{k}", name=f"wkk{k}") for k in range(NKT)]
                        wvk = [wp.tile([128, 2 * HD], F32R, tag=f"wv{k}", name=f"wvk{k}") for k in range(NKT)]
                        # DMA order: per-k weights interleaved with chunk-0 x
                        # tiles so the first matmuls can start ~2us in.
                        xk0 = []
                        for k in range(NKT):
                            r = slice(k * 128, (k + 1) * 128)
                            nc.sync.dma_start(wqk[k][:], wq[r, :])
                            nc.sync.dma_start(wkk[k][:], wk[r, :])
                            nc.sync.dma_start(wvk[k][:], wv[r, :])
                            if k < 8:
                                xk = xp.tile([128, 512], F32R, tag="xk", name=f"xk0_{k}")
                                nc.sync.dma_start(xk[:], xT[r, 0:512])
                                xk0.append(xk)
                        nc.sync.dma_start(protb[:], prot[:])
                        nc.sync.dma_start(cosb[:], cosT2[:])
                        nc.sync.dma_start(sinb[:], sinT2[:])
                        nc.sync.dma_start(trib[:], tri[:])
                        nc.sync.dma_start(identb[:], ident[:])
                        nc.sync.dma_start(selAb[:], selA[:])
                        nc.sync.dma_start(selBb[:], selB[:])
                        nc.sync.dma_start(onesb[:], onescol[:])
                        nc.sync.dma_start(zblkb[:], zblk[:])

                        def emit_rope(n):
                            ncol = slice(n * 512, (n + 1) * 512)
                            for tl in [*qT, kT]:
                                rps = rpp.tile([128, 512], F32, tag="rps")
                                nc.tensor.matmul(rps[:], protb[:], tl[:, ncol],
                                                 start=True, stop=True)
                                tmp = rtp.tile([128, 512], F32, tag="ropetmp")
                                nc.vector.tensor_mul(tmp[:], tl[:, ncol], cosb[:, ncol])
                                nc.vector.tensor_mul(rps[:], rps[:], sinb[:, ncol])
                                nc.vector.tensor_add(tl[:, ncol], tmp[:], rps[:])

                        for n in range(NCHUNK):
                            ncol = slice(n * 512, (n + 1) * 512)
                            accs = [pqkv.tile([128, 512], F32, tag="qkvacc", name=f"acc{n}_{m}") for m in range(6)]
                            for k in range(NKT):
                                if n == 0 and k < 8:
                                    xk = xk0[k]
                                else:
                                    xk = xp.tile([128, 512], F32R, tag="xk")
                                    nc.sync.dma_start(xk[:], xT[k * 128:(k + 1) * 128, ncol])
                                st = k == 0
                                sp = k == NKT - 1
                                for t in range(4):
                                    nc.tensor.matmul(
                                        accs[t][:], wqk[k][:, t * 128:(t + 1) * 128],
                                        xk[:], start=st, stop=sp)
                                nc.tensor.matmul(accs[4][:], wkk[k][:], xk[:], start=st, stop=sp)
                                nc.tensor.matmul(accs[5][:], wvk[k][:], xk[:], start=st, stop=sp)
                            # drains alternate DVE / Act so accs free quickly
                            for m, tl in enumerate([*qT, kT]):
                                if m % 2 == 0:
                                    nc.vector.tensor_copy(tl[:, ncol], accs[m][:])
                                else:
                                    nc.scalar.copy(tl[:, ncol], accs[m][:])
                            nc.scalar.copy(vT[:, ncol], accs[5][:])
                            if n == 0:
                                for k4 in range(4):
                                    nc.sync.dma_start(
                                        wotp[:, k4 * D:(k4 + 1) * D],
                                        wo[k4 * 128:(k4 + 1) * 128, :])
                            # RoPE one chunk behind: its rot matmul depends on
                            # the drain above, so running it inside the next
                            # chunk's matmul stream keeps the PE busy.
                            if n > 0:
                                emit_rope(n - 1)
                        emit_rope(NCHUNK - 1)

                    # ---------- Phase C: v natural tiles [128, 65] bf16 ----------
                    with tc.tile_pool(name="vtp", bufs=2, space="PSUM") as vtp:
                        for i in range(NST):
                            for g in range(2):
                                vps = vtp.tile([128, 64], F32R, tag="vps")
                                nc.tensor.transpose(
                                    vps[:], vT[g * 64:(g + 1) * 64, i * 128:(i + 1) * 128],
                                    identb[g * 64:(g + 1) * 64, :])
                                vt = vp.tile([128, 65], BF16, tag=f"vo{g}_{i}", name=f"vo{g}_{i}")
                                nc.vector.tensor_copy(vt[:, 0:64], vps[:])
                                nc.vector.tensor_copy(vt[:, 64:65], onesb[:])
                                vo[g][i] = vt

                # ---------- Phase D: attention + fused normalize/o_proj ----------
                with (
                    tc.tile_pool(name="esb", bufs=18) as ep,
                    tc.tile_pool(name="rcpp", bufs=2) as rcp_,
                    tc.tile_pool(name="oout", bufs=3) as op,
                    tc.tile_pool(name="sps", bufs=2, space="PSUM") as sp_,
                    tc.tile_pool(name="avp", bufs=2, space="PSUM") as ap_,
                    tc.tile_pool(name="bcps", bufs=1, space="PSUM") as bp_,
                    tc.tile_pool(name="ops", bufs=1, space="PSUM") as opp,
                ):
                    backlog = deque()

                    def drain_backlog(k):
                        while backlog and k:
                            backlog.popleft()()
                            k -= 1

                    for j in range(NCHUNK):
                        jcol = slice(j * 512, (j + 1) * 512)
                        ni = 4 * j + 4
                        nh = ni // 2
                        for t in range(4):
                            for sub in range(2):
                                pb = slice(64 * sub, 64 * sub + 64)
                                g = sub
                                avs = ap_.tile([65, 512], F32, tag="avacc",
                                               name=f"av{t}_{j}_{sub}")
                                esl = []
                                # score/exp chain: two key blocks per PSUM
                                # tile, one wide exp each (Act-paced)
                                for h in range(nh):
                                    ssq = sp_.tile([128, 1024], F32, tag="scps")
                                    for s_ in range(2):
                                        i = 2 * h + s_
                                        c0 = max(0, 128 * (i - 4 * j))
                                        ec0 = c0 if 512 - c0 >= 256 else 256
                                        nc.tensor.matmul(
                                            ssq[:, s_ * 512 + ec0:(s_ + 1) * 512],
                                            kT[pb, i * 128:(i + 1) * 128],
                                            qT[t][pb, j * 512 + ec0:(j + 1) * 512],
                                            start=True, stop=True)
                                    es = ep.tile([128, 1024], BF16, tag="es")
                                    nc.scalar.activation(es[:], ssq[:], EXP, scale=SCALE)
                                    for s_ in range(2):
                                        i = 2 * h + s_
                                        c0 = max(0, 128 * (i - 4 * j))
                                        if i >= 4 * j:
                                            nc.vector.tensor_mul(
                                                es[:, s_ * 512 + c0:s_ * 512 + c0 + 128],
                                                es[:, s_ * 512 + c0:s_ * 512 + c0 + 128],
                                                trib[:])
                                        if c0 == 384:
                                            nc.vector.tensor_copy(
                                                es[:, s_ * 512 + 256:s_ * 512 + 384],
                                                zblkb[:])
                                    esl.append(es)
                                    drain_backlog(1)

                                # AV matmuls: one same-config block, run one
                                # pass behind via the backlog
                                def make_av(t=t, sub=sub, g=g, avs=avs, esl=esl,
                                            jv=j, niv=ni, pbv=pb, jc=jcol):
                                    def av_block(i0, i1):
                                        def emit():
                                            for i in range(i0, i1):
                                                c0 = max(0, 128 * (i - 4 * jv))
                                                av0 = c0 if c0 < 384 else 256
                                                s_ = i % 2
                                                es = esl[i // 2]
                                                nc.tensor.matmul(
                                                    avs[:, av0:512], vo[g][i][:],
                                                    es[:, s_ * 512 + av0:(s_ + 1) * 512],
                                                    start=(i == 0), stop=(i == niv - 1))
                                        return emit

                                    def drain():
                                        nc.vector.tensor_copy(qT[t][pbv, jc], avs[0:64, :])
                                        dst = denomA if sub == 0 else denomB
                                        nc.vector.tensor_copy(
                                            dst[32 * t:32 * t + 1, jc], avs[64:65, :])
                                    items = [av_block(i0, min(i0 + 4, niv))
                                             for i0 in range(0, niv, 4)]
                                    items.append(drain)
                                    return items
                                backlog.extend(make_av())

                        # flush: last pass's AVs must land before normalize
                        drain_backlog(len(backlog))

                        # normalize chunk j across all heads
                        nc.vector.reciprocal_approx_fast(
                            out=denomA[:, jcol], in_=denomA[:, jcol])
                        nc.vector.reciprocal_approx_fast(
                            out=denomB[:, jcol], in_=denomB[:, jcol])
                        rA = rcp_.tile([128, 512], F32R, tag="rA")
                        rB = rcp_.tile([128, 512], F32R, tag="rB")
                        nc.vector.tensor_copy(rA[:], denomA[:, jcol])
                        nc.vector.tensor_copy(rB[:], denomB[:, jcol])
                        for t in range(4):
                            tsl = slice(t * 128, (t + 1) * 128)
                            bps = bp_.tile([128, 512], F32, tag="bps")
                            nc.tensor.matmul(bps[:], selAb[:, tsl], rA[:],
                                             start=True, stop=False)
                            nc.tensor.matmul(bps[:], selBb[:, tsl], rB[:],
                                             start=False, stop=True)
                            nc.vector.tensor_mul(qT[t][:, jcol], qT[t][:, jcol], bps[:])

                        # o_proj rows for this chunk -> backlog (run during
                        # the next chunk's Act-paced score phase)
                        def make_oproj(jv=j):
                            items = []
                            for st in range(4 * jv, 4 * jv + 4):
                                stc = slice(st * 128, (st + 1) * 128)
                                for dch in range(NCHUNK):
                                    def emit(st=st, stc=stc, dch=dch):
                                        ops = opp.tile([128, 512], F32, tag="opps")
                                        for kt in range(4):
                                            nc.tensor.matmul(
                                                ops[:], qT[kt][:, stc],
                                                wotp[:, kt * D + dch * 512:kt * D + (dch + 1) * 512],
                                                start=(kt == 0), stop=(kt == 3))
                                        oo = op.tile([128, 512], F32, tag="oo")
                                        nc.vector.tensor_copy(oo[:], ops[:])
                                        nc.sync.dma_start(
                                            opart[stc, dch * 512:(dch + 1) * 512],
                                            oo[:])
                                    items.append(emit)
                            return items
                        backlog.extend(make_oproj())

                    drain_backlog(len(backlog))

    nc.compile()
    return nc


_PROGRAM = None


def _get_program():
    global _PROGRAM
    if _PROGRAM is None:
        _PROGRAM = _build_program()
    return _PROGRAM


def _make_in_maps(x, cos, sin, Wq, Wk, Wv, Wo):
    cosT = np.ascontiguousarray(cos.T.astype(np.float32))      # [64, S]
    sinT = np.ascontiguousarray(sin.T.astype(np.float32))
    cosT2 = np.tile(cosT, (2, 1))
    sinT2 = np.tile(sinT, (2, 1))
    tri = (np.arange(128)[None, :] >= np.arange(128)[:, None])
    tri = tri.astype(np.float32).astype(np.dtype("bfloat16") if hasattr(np, "bfloat16") else np.float32)
    import ml_dtypes
    tri = (np.arange(128)[None, :] >= np.arange(128)[:, None]).astype(ml_dtypes.bfloat16)
    ident = np.tile(np.eye(64, dtype=np.float32), (2, 1))
    # rotate-half permutation (sign folded): rot[m] = -tl[m+32] (m%64<32),
    # rot[m] = +tl[m-32] (m%64>=32); out[m,n] = sum_k prot[k,m]*tl[k,n]
    prot = np.zeros((128, 128), dtype=np.float32)
    for m in range(128):
        if m % 64 < 32:
            prot[m + 32, m] = -1.0
        else:
            prot[m - 32, m] = 1.0
    selA = np.zeros((128, 512), dtype=np.float32)
    selB = np.zeros((128, 512), dtype=np.float32)
    for t in range(4):
        selA[32 * t, 128 * t:128 * t + 64] = 1.0
        selB[32 * t, 128 * t + 64:128 * t + 128] = 1.0

    perm = [0, 4, 1, 5, 2, 6, 3, 7]
    in_maps = []
    for c in range(8):
        b, q = c // 4, c % 4
        idx = np.concatenate([np.arange(HD) + (8 * q + j) * HD for j in perm])
        in_maps.append({
            "xT": np.ascontiguousarray(x[b].T.astype(np.float32)),
            "wq": np.ascontiguousarray(Wq[:, idx].astype(np.float32)),
            "wk": np.ascontiguousarray(Wk[:, 2 * q * HD:(2 * q + 2) * HD].astype(np.float32)),
            "wv": np.ascontiguousarray(Wv[:, 2 * q * HD:(2 * q + 2) * HD].astype(np.float32)),
            "wo": np.ascontiguousarray(Wo[idx, :].astype(np.float32)),
            "cosT2": cosT2,
            "sinT2": sinT2,
            "prot": prot,
            "tri": tri,
            "ident": ident,
            "selA": selA,
            "selB": selB,
            "onescol": np.ones((128, 1), dtype=ml_dtypes.bfloat16),
            "zblk": np.zeros((128, 128), dtype=ml_dtypes.bfloat16),
        })
    return in_maps


def _execute(in_maps, trace=False):
    nc = _get_program()
    return bass_utils.run_bass_kernel_spmd(
        nc, in_maps, core_ids=list(range(8)), trace=trace)


def kernel(x, cos, sin, Wq, Wk, Wv, Wo):
    in_maps = _make_in_maps(x, cos, sin, Wq, Wk, Wv, Wo)
    res = _execute(in_maps, trace=False)
    parts = [r["opart"] for r in res.results]
    out = np.empty((B, S, D), dtype=np.float32)
    for b in range(B):
        p = parts[4 * b:4 * b + 4]
        out[b] = (p[0] + p[1]) + (p[2] + p[3])
    return out


# revision 9
# speedup vs baseline: 1.7655x; 1.0245x over previous
"""GQA (RoPE + causal softmax) Trainium2 Bass kernel, 8-core SPMD.

Sharding: DP over batch (2) x TP over KV groups (4 quarters of heads).
Core c handles batch c//4 and head quarter c%4 (8 q-heads, 2 kv-heads).
Each core computes a partial o_proj ([S, D]); host sums 4 partials per batch.

Matmuls run in float32r (1 col/cyc at N>=256, PE @2.4GHz); attention
probabilities and V are bf16. Everything on-chip is in "transposed"
layout (feature dim on partitions), so x^T is the only host-side prep.

Schedule (PE and Act kept concurrently busy):
 - Phase A: qkv^T = W^T @ x^T per 512-col chunk. Chunk-0 x DMAs are
   interleaved with the weight DMAs so the first matmul starts ~2us in.
   RoPE for chunk n (PE permutation matmul + DVE mul/mul/add) is
   emitted after chunk n+1's projection matmuls so it never stalls the
   PE. PSUM drains alternate DVE/Act.
 - Phase C: v natural tiles [128, 65] bf16 (ones column -> denominator
   rides the AV matmul).
 - Phase D (j-outer): scores for two key blocks go into one [128,1024]
   PSUM tile and get a single wide exp into a persistent bf16 SBUF
   tile. The score->exp chain is Act-paced; the PE's AV matmuls (one
   same-config block per head-pass, run one pass behind) and o_proj
   tiles (one chunk behind) are drawn from a backlog to fill the PE
   during Act waits. Chunk normalization uses reciprocal_approx_fast
   and a select-matmul partition broadcast; attention output reuses
   the qT tiles (their columns are dead after the chunk's scores).
"""

from collections import deque

import numpy as np

import concourse.bass as bass
import concourse.mybir as mybir
import concourse.tile as tile
from concourse import bacc, bass_utils

B, S, D = 2, 2048, 2048
H, KV, HD = 32, 8, 64
REP = H // KV
SCALE = 1.0 / 8.0  # 1/sqrt(HD)

F32 = mybir.dt.float32
F32R = mybir.dt.float32r
BF16 = mybir.dt.bfloat16
EXP = mybir.ActivationFunctionType.Exp

NCHUNK = S // 512        # 4 sq chunks of 512
NKT = D // 128           # 16 k-tiles over D
NST = S // 128           # 16 sk/s tiles


def _build_program():
    nc = bacc.Bacc()

    xT = nc.dram_tensor("xT", [D, S], F32R, kind="ExternalInput").ap()
    wq = nc.dram_tensor("wq", [D, 8 * HD], F32R, kind="ExternalInput").ap()
    wk = nc.dram_tensor("wk", [D, 2 * HD], F32R, kind="ExternalInput").ap()
    wv = nc.dram_tensor("wv", [D, 2 * HD], F32R, kind="ExternalInput").ap()
    wo = nc.dram_tensor("wo", [8 * HD, D], F32R, kind="ExternalInput").ap()
    cosT2 = nc.dram_tensor("cosT2", [128, S], F32, kind="ExternalInput").ap()
    sinT2 = nc.dram_tensor("sinT2", [128, S], F32, kind="ExternalInput").ap()
    prot = nc.dram_tensor("prot", [128, 128], F32R, kind="ExternalInput").ap()
    tri = nc.dram_tensor("tri", [128, 128], BF16, kind="ExternalInput").ap()
    ident = nc.dram_tensor("ident", [128, 64], F32R, kind="ExternalInput").ap()
    selA = nc.dram_tensor("selA", [128, 512], F32R, kind="ExternalInput").ap()
    selB = nc.dram_tensor("selB", [128, 512], F32R, kind="ExternalInput").ap()
    onescol = nc.dram_tensor("onescol", [128, 1], BF16, kind="ExternalInput").ap()
    zblk = nc.dram_tensor("zblk", [128, 128], BF16, kind="ExternalInput").ap()
    opart = nc.dram_tensor("opart", [S, D], F32, kind="ExternalOutput").ap()

    with tile.TileContext(nc) as tc:
        with (
            tc.tile_pool(name="persist", bufs=1) as pp,
            tc.tile_pool(name="consts", bufs=1) as cp,
        ):
            # persistent SBUF: q^T/k^T (qT doubles as attn-out storage),
            # denominators, small constants
            qT = [pp.tile([128, S], F32R, tag=f"qT{t}", name=f"qT{t}") for t in range(4)]
            kT = pp.tile([128, S], F32R, tag="kT")
            denomA = pp.tile([128, S], F32, tag="denomA")
            denomB = pp.tile([128, S], F32, tag="denomB")
            cosb = cp.tile([128, S], F32, tag="cosb")
            sinb = cp.tile([128, S], F32, tag="sinb")
            protb = cp.tile([128, 128], F32R, tag="protb")
            trib = cp.tile([128, 128], BF16, tag="trib")
            identb = cp.tile([128, 64], F32R, tag="identb")
            selAb = cp.tile([128, 512], F32R, tag="selAb")
            selBb = cp.tile([128, 512], F32R, tag="selBb")
            onesb = cp.tile([128, 1], BF16, tag="onesb")
            zblkb = cp.tile([128, 128], BF16, tag="zblkb")
            wotp = pp.tile([128, 4 * D], F32R, tag="wotp")  # wo tiles, concat
            nc.gpsimd.memset(denomA[:], 1.0)
            nc.gpsimd.memset(denomB[:], 1.0)

            vo = [[None] * NST, [None] * NST]
            with tc.tile_pool(name="vop", bufs=1) as vp:  # spans C..D
                with (
                    tc.tile_pool(name="vtbuf", bufs=1) as vtb,
                    tc.tile_pool(name="rotps", bufs=2, space="PSUM") as rpp,
                    tc.tile_pool(name="ropet", bufs=2) as rtp,
                ):
                    vT = vtb.tile([128, S], F32R, tag="vT")

                    def emit_rope(n):
                        ncol = slice(n * 512, (n + 1) * 512)
                        for tl in [*qT, kT]:
                            rps = rpp.tile([128, 512], F32, tag="rps")
                            nc.tensor.matmul(rps[:], protb[:], tl[:, ncol],
                                             start=True, stop=True)
                            tmp = rtp.tile([128, 512], F32, tag="ropetmp")
                            nc.vector.tensor_mul(tmp[:], tl[:, ncol], cosb[:, ncol])
                            nc.vector.tensor_mul(rps[:], rps[:], sinb[:, ncol])
                            nc.vector.tensor_add(tl[:, ncol], tmp[:], rps[:])

                    # ---------- Phase A: qkv^T = W^T @ x^T, + RoPE ----------
                    with (
                        tc.tile_pool(name="wts", bufs=1) as wp,
                        tc.tile_pool(name="xin", bufs=12) as xp,
                        tc.tile_pool(name="qkvps", bufs=6, space="PSUM") as pqkv,
                    ):
                        wqk = [wp.tile([128, 8 * HD], F32R, tag=f"wq{k}", name=f"wqk{k}") for k in range(NKT)]
                        wkk = [wp.tile([128, 2 * HD], F32R, tag=f"wk{k}", name=f"wkk{k}") for k in range(NKT)]
                        wvk = [wp.tile([128, 2 * HD], F32R, tag=f"wv{k}", name=f"wvk{k}") for k in range(NKT)]
                        # DMA order: per-k weights interleaved with chunk-0 x
                        # tiles so the first matmuls can start ~2us in.
                        xk0 = []
                        for k in range(NKT):
                            r = slice(k * 128, (k + 1) * 128)
                            nc.sync.dma_start(wqk[k][:], wq[r, :])
                            nc.sync.dma_start(wkk[k][:], wk[r, :])
                            nc.sync.dma_start(wvk[k][:], wv[r, :])
                            if k < 8:
                                xk = xp.tile([128, 512], F32R, tag="xk", name=f"xk0_{k}")
                                nc.sync.dma_start(xk[:], xT[r, 0:512])
                                xk0.append(xk)
                        nc.sync.dma_start(protb[:], prot[:])
                        nc.sync.dma_start(cosb[:], cosT2[:])
                        nc.sync.dma_start(sinb[:], sinT2[:])
                        nc.sync.dma_start(trib[:], tri[:])
                        nc.sync.dma_start(identb[:], ident[:])
                        nc.sync.dma_start(selAb[:], selA[:])
                        nc.sync.dma_start(selBb[:], selB[:])
                        nc.sync.dma_start(onesb[:], onescol[:])
                        nc.sync.dma_start(zblkb[:], zblk[:])

                        for n in range(NCHUNK):
                            ncol = slice(n * 512, (n + 1) * 512)
                            accs = [pqkv.tile([128, 512], F32, tag="qkvacc", name=f"acc{n}_{m}") for m in range(6)]
                            for k in range(NKT):
                                if n == 0 and k < 8:
                                    xk = xk0[k]
                                else:
                                    xk = xp.tile([128, 512], F32R, tag="xk")
                                    nc.sync.dma_start(xk[:], xT[k * 128:(k + 1) * 128, ncol])
                                st = k == 0
                                sp = k == NKT - 1
                                for t in range(4):
                                    nc.tensor.matmul(
                                        accs[t][:], wqk[k][:, t * 128:(t + 1) * 128],
                                        xk[:], start=st, stop=sp)
                                nc.tensor.matmul(accs[4][:], wkk[k][:], xk[:], start=st, stop=sp)
                                nc.tensor.matmul(accs[5][:], wvk[k][:], xk[:], start=st, stop=sp)
                            # drains alternate DVE / Act so accs free quickly
                            for m, tl in enumerate([*qT, kT]):
                                if m % 2 == 0:
                                    nc.vector.tensor_copy(tl[:, ncol], accs[m][:])
                                else:
                                    nc.scalar.copy(tl[:, ncol], accs[m][:])
                            nc.scalar.copy(vT[:, ncol], accs[5][:])
                            if n == 2:
                                for k4 in range(4):
                                    nc.sync.dma_start(
                                        wotp[:, k4 * D:(k4 + 1) * D],
                                        wo[k4 * 128:(k4 + 1) * 128, :])
                            # RoPE one chunk behind: its rot matmul depends on
                            # the drain above, so running it inside the next
                            # chunk's matmul stream keeps the PE busy.
                            if n > 0:
                                emit_rope(n - 1)

                    # ---------- Phase C: v natural tiles [128, 65] bf16 ----------
                    with tc.tile_pool(name="vtp", bufs=2, space="PSUM") as vtp:
                        for i in range(NST):
                            for g in range(2):
                                vps = vtp.tile([128, 64], F32R, tag="vps")
                                nc.tensor.transpose(
                                    vps[:], vT[g * 64:(g + 1) * 64, i * 128:(i + 1) * 128],
                                    identb[g * 64:(g + 1) * 64, :])
                                vt = vp.tile([128, 65], BF16, tag=f"vo{g}_{i}", name=f"vo{g}_{i}")
                                nc.vector.tensor_copy(vt[:, 0:64], vps[:])
                                nc.vector.tensor_copy(vt[:, 64:65], onesb[:])
                                vo[g][i] = vt
                        emit_rope(NCHUNK - 1)

                # ---------- Phase D: attention + fused normalize/o_proj ----------
                with (
                    tc.tile_pool(name="esb", bufs=18) as ep,
                    tc.tile_pool(name="rcpp", bufs=2) as rcp_,
                    tc.tile_pool(name="oout", bufs=3) as op,
                    tc.tile_pool(name="sps", bufs=2, space="PSUM") as sp_,
                    tc.tile_pool(name="avp", bufs=2, space="PSUM") as ap_,
                    tc.tile_pool(name="bcps", bufs=1, space="PSUM") as bp_,
                    tc.tile_pool(name="ops", bufs=1, space="PSUM") as opp,
                ):
                    backlog = deque()

                    def drain_backlog(k):
                        while backlog and k:
                            backlog.popleft()()
                            k -= 1

                    for j in range(NCHUNK):
                        jcol = slice(j * 512, (j + 1) * 512)
                        ni = 4 * j + 4
                        nh = ni // 2
                        for t in range(4):
                            for sub in range(2):
                                pb = slice(64 * sub, 64 * sub + 64)
                                g = sub
                                avs = ap_.tile([65, 512], F32, tag="avacc",
                                               name=f"av{t}_{j}_{sub}")
                                esl = []
                                # score/exp chain: two key blocks per PSUM
                                # tile, one wide exp each (Act-paced)
                                for h in range(nh):
                                    ssq = sp_.tile([128, 1024], F32, tag="scps")
                                    for s_ in range(2):
                                        i = 2 * h + s_
                                        c0 = max(0, 128 * (i - 4 * j))
                                        ec0 = c0 if 512 - c0 >= 256 else 256
                                        nc.tensor.matmul(
                                            ssq[:, s_ * 512 + ec0:(s_ + 1) * 512],
                                            kT[pb, i * 128:(i + 1) * 128],
                                            qT[t][pb, j * 512 + ec0:(j + 1) * 512],
                                            start=True, stop=True)
                                    es = ep.tile([128, 1024], BF16, tag="es")
                                    nc.scalar.activation(es[:], ssq[:], EXP, scale=SCALE)
                                    for s_ in range(2):
                                        i = 2 * h + s_
                                        c0 = max(0, 128 * (i - 4 * j))
                                        if i >= 4 * j:
                                            nc.vector.tensor_mul(
                                                es[:, s_ * 512 + c0:s_ * 512 + c0 + 128],
                                                es[:, s_ * 512 + c0:s_ * 512 + c0 + 128],
                                                trib[:])
                                        if c0 == 384:
                                            nc.vector.tensor_copy(
                                                es[:, s_ * 512 + 256:s_ * 512 + 384],
                                                zblkb[:])
                                    esl.append(es)
                                    drain_backlog(1)

                                # AV matmuls: one same-config block, run one
                                # pass behind via the backlog
                                def make_av(t=t, sub=sub, g=g, avs=avs, esl=esl,
                                            jv=j, niv=ni, pbv=pb, jc=jcol):
                                    def av_block(i0, i1):
                                        def emit():
                                            for i in range(i0, i1):
                                                c0 = max(0, 128 * (i - 4 * jv))
                                                av0 = c0 if c0 < 384 else 256
                                                s_ = i % 2
                                                es = esl[i // 2]
                                                nc.tensor.matmul(
                                                    avs[:, av0:512], vo[g][i][:],
                                                    es[:, s_ * 512 + av0:(s_ + 1) * 512],
                                                    start=(i == 0), stop=(i == niv - 1))
                                        return emit

                                    def drain():
                                        nc.vector.tensor_copy(qT[t][pbv, jc], avs[0:64, :])
                                        dst = denomA if sub == 0 else denomB
                                        nc.vector.tensor_copy(
                                            dst[32 * t:32 * t + 1, jc], avs[64:65, :])
                                    items = [av_block(i0, min(i0 + 4, niv))
                                             for i0 in range(0, niv, 4)]
                                    items.append(drain)
                                    return items
                                backlog.extend(make_av())

                        # flush: last pass's AVs must land before normalize
                        drain_backlog(len(backlog))

                        # normalize chunk j across all heads
                        nc.vector.reciprocal_approx_fast(
                            out=denomA[:, jcol], in_=denomA[:, jcol])
                        nc.vector.reciprocal_approx_fast(
                            out=denomB[:, jcol], in_=denomB[:, jcol])
                        rA = rcp_.tile([128, 512], F32R, tag="rA")
                        rB = rcp_.tile([128, 512], F32R, tag="rB")
                        nc.vector.tensor_copy(rA[:], denomA[:, jcol])
                        nc.vector.tensor_copy(rB[:], denomB[:, jcol])
                        for t in range(4):
                            tsl = slice(t * 128, (t + 1) * 128)
                            bps = bp_.tile([128, 512], F32, tag="bps")
                            nc.tensor.matmul(bps[:], selAb[:, tsl], rA[:],
                                             start=True, stop=False)
                            nc.tensor.matmul(bps[:], selBb[:, tsl], rB[:],
                                             start=False, stop=True)
                            nc.vector.tensor_mul(qT[t][:, jcol], qT[t][:, jcol], bps[:])

                        # o_proj rows for this chunk -> backlog (run during
                        # the next chunk's Act-paced score phase)
                        def make_oproj(jv=j):
                            items = []
                            for st in range(4 * jv, 4 * jv + 4):
                                stc = slice(st * 128, (st + 1) * 128)
                                for dch in range(NCHUNK):
                                    def emit(st=st, stc=stc, dch=dch):
                                        ops = opp.tile([128, 512], F32, tag="opps")
                                        for kt in range(4):
                                            nc.tensor.matmul(
                                                ops[:], qT[kt][:, stc],
                                                wotp[:, kt * D + dch * 512:kt * D + (dch + 1) * 512],
                                                start=(kt == 0), stop=(kt == 3))
                                        oo = op.tile([128, 512], F32, tag="oo")
                                        nc.vector.tensor_copy(oo[:], ops[:])
                                        nc.sync.dma_start(
                                            opart[stc, dch * 512:(dch + 1) * 512],
                                            oo[:])
                                    items.append(emit)
                            return items
                        backlog.extend(make_oproj())

                    drain_backlog(len(backlog))

    nc.compile()
    return nc


_PROGRAM = None


def _get_program():
    global _PROGRAM
    if _PROGRAM is None:
        _PROGRAM = _build_program()
    return _PROGRAM


def _make_in_maps(x, cos, sin, Wq, Wk, Wv, Wo):
    cosT = np.ascontiguousarray(cos.T.astype(np.float32))      # [64, S]
    sinT = np.ascontiguousarray(sin.T.astype(np.float32))
    cosT2 = np.tile(cosT, (2, 1))
    sinT2 = np.tile(sinT, (2, 1))
    tri = (np.arange(128)[None, :] >= np.arange(128)[:, None])
    tri = tri.astype(np.float32).astype(np.dtype("bfloat16") if hasattr(np, "bfloat16") else np.float32)
    import ml_dtypes
    tri = (np.arange(128)[None, :] >= np.arange(128)[:, None]).astype(ml_dtypes.bfloat16)
    ident = np.tile(np.eye(64, dtype=np.float32), (2, 1))
    # rotate-half permutation (sign folded): rot[m] = -tl[m+32] (m%64<32),
    # rot[m] = +tl[m-32] (m%64>=32); out[m,n] = sum_k prot[k,m]*tl[k,n]
    prot = np.zeros((128, 128), dtype=np.float32)
    for m in range(128):
        if m % 64 < 32:
            prot[m + 32, m] = -1.0
        else:
            prot[m - 32, m] = 1.0
    selA = np.zeros((128, 512), dtype=np.float32)
    selB = np.zeros((128, 512), dtype=np.float32)
    for t in range(4):
        selA[32 * t, 128 * t:128 * t + 64] = 1.0
        selB[32 * t, 128 * t + 64:128 * t + 128] = 1.0

    perm = [0, 4, 1, 5, 2, 6, 3, 7]
    in_maps = []
    for c in range(8):
        b, q = c // 4, c % 4
        idx = np.concatenate([np.arange(HD) + (8 * q + j) * HD for j in perm])
        in_maps.append({
            "xT": np.ascontiguousarray(x[b].T.astype(np.float32)),
            "wq": np.ascontiguousarray(Wq[:, idx].astype(np.float32)),
            "wk": np.ascontiguousarray(Wk[:, 2 * q * HD:(2 * q + 2) * HD].astype(np.float32)),
            "wv": np.ascontiguousarray(Wv[:, 2 * q * HD:(2 * q + 2) * HD].astype(np.float32)),
            "wo": np.ascontiguousarray(Wo[idx, :].astype(np.float32)),
            "cosT2": cosT2,
            "sinT2": sinT2,
            "prot": prot,
            "tri": tri,
            "ident": ident,
            "selA": selA,
            "selB": selB,
            "onescol": np.ones((128, 1), dtype=ml_dtypes.bfloat16),
            "zblk": np.zeros((128, 128), dtype=ml_dtypes.bfloat16),
        })
    return in_maps


def _execute(in_maps, trace=False):
    nc = _get_program()
    return bass_utils.run_bass_kernel_spmd(
        nc, in_maps, core_ids=list(range(8)), trace=trace)


def kernel(x, cos, sin, Wq, Wk, Wv, Wo):
    in_maps = _make_in_maps(x, cos, sin, Wq, Wk, Wv, Wo)
    res = _execute(in_maps, trace=False)
    parts = [r["opart"] for r in res.results]
    out = np.empty((B, S, D), dtype=np.float32)
    for b in range(B):
        p = parts[4 * b:4 * b + 4]
        out[b] = (p[0] + p[1]) + (p[2] + p[3])
    return out


# revision 13
# speedup vs baseline: 1.7791x; 1.0077x over previous
"""GQA (RoPE + causal softmax) Trainium2 Bass kernel, 8-core SPMD.

Sharding: DP over batch (2) x TP over KV groups (4 quarters of heads).
Core c handles batch c//4 and head quarter c%4 (8 q-heads, 2 kv-heads).
Each core computes a partial o_proj ([S, D]); host sums 4 partials per batch.

Matmuls run in float32r (1 col/cyc at N>=256, PE @2.4GHz); attention
probabilities and V are bf16. Everything on-chip is in "transposed"
layout (feature dim on partitions), so x^T is the only host-side prep.

Schedule (PE and Act kept concurrently busy):
 - Phase A: qkv^T = W^T @ x^T per 512-col chunk. Chunk-0 x DMAs are
   interleaved with the weight DMAs so the first matmul starts ~2us in.
   RoPE for chunk n (PE permutation matmul + DVE mul/mul/add) is
   emitted after chunk n+1's projection matmuls so it never stalls the
   PE. PSUM drains alternate DVE/Act.
 - Phase C: v natural tiles [128, 65] bf16 (ones column -> denominator
   rides the AV matmul).
 - Phase D (j-outer): scores for two key blocks go into one [128,1024]
   PSUM tile and get a single wide exp into a persistent bf16 SBUF
   tile. The score->exp chain is Act-paced; the PE's AV matmuls (one
   same-config block per head-pass, run one pass behind) and o_proj
   tiles (one chunk behind) are drawn from a backlog to fill the PE
   during Act waits. Chunk normalization uses reciprocal_approx_fast
   and a select-matmul partition broadcast; attention output reuses
   the qT tiles (their columns are dead after the chunk's scores).
"""

from collections import deque

import numpy as np

import concourse.bass as bass
import concourse.mybir as mybir
import concourse.tile as tile
from concourse import bacc, bass_utils

B, S, D = 2, 2048, 2048
H, KV, HD = 32, 8, 64
REP = H // KV
SCALE = 1.0 / 8.0  # 1/sqrt(HD)

F32 = mybir.dt.float32
F32R = mybir.dt.float32r
BF16 = mybir.dt.bfloat16
EXP = mybir.ActivationFunctionType.Exp

NCHUNK = S // 512        # 4 sq chunks of 512
NKT = D // 128           # 16 k-tiles over D
NST = S // 128           # 16 sk/s tiles


def _build_program():
    nc = bacc.Bacc()

    xT = nc.dram_tensor("xT", [D, S], F32R, kind="ExternalInput").ap()
    wq = nc.dram_tensor("wq", [D, 8 * HD], F32R, kind="ExternalInput").ap()
    wk = nc.dram_tensor("wk", [D, 2 * HD], F32R, kind="ExternalInput").ap()
    wv = nc.dram_tensor("wv", [D, 2 * HD], F32R, kind="ExternalInput").ap()
    wo = nc.dram_tensor("wo", [8 * HD, D], F32R, kind="ExternalInput").ap()
    cosT2 = nc.dram_tensor("cosT2", [128, S], F32, kind="ExternalInput").ap()
    sinT2 = nc.dram_tensor("sinT2", [128, S], F32, kind="ExternalInput").ap()
    prot = nc.dram_tensor("prot", [128, 128], F32R, kind="ExternalInput").ap()
    tri = nc.dram_tensor("tri", [128, 128], BF16, kind="ExternalInput").ap()
    ident = nc.dram_tensor("ident", [128, 64], F32R, kind="ExternalInput").ap()
    selA = nc.dram_tensor("selA", [128, 512], F32R, kind="ExternalInput").ap()
    selB = nc.dram_tensor("selB", [128, 512], F32R, kind="ExternalInput").ap()
    onescol = nc.dram_tensor("onescol", [128, 1], BF16, kind="ExternalInput").ap()
    zblk = nc.dram_tensor("zblk", [128, 128], BF16, kind="ExternalInput").ap()
    opart = nc.dram_tensor("opart", [S, D], F32, kind="ExternalOutput").ap()

    with tile.TileContext(nc) as tc:
        with (
            tc.tile_pool(name="persist", bufs=1) as pp,
            tc.tile_pool(name="consts", bufs=1) as cp,
        ):
            # persistent SBUF: q^T/k^T (qT doubles as attn-out storage),
            # denominators, small constants
            qT = [pp.tile([128, S], F32R, tag=f"qT{t}", name=f"qT{t}") for t in range(4)]
            kT = pp.tile([128, S], F32R, tag="kT")
            denomA = pp.tile([128, S], F32, tag="denomA")
            denomB = pp.tile([128, S], F32, tag="denomB")
            cosb = cp.tile([128, S], F32, tag="cosb")
            sinb = cp.tile([128, S], F32, tag="sinb")
            protb = cp.tile([128, 128], F32R, tag="protb")
            trib = cp.tile([128, 128], BF16, tag="trib")
            identb = cp.tile([128, 64], F32R, tag="identb")
            selAb = cp.tile([128, 512], F32R, tag="selAb")
            selBb = cp.tile([128, 512], F32R, tag="selBb")
            onesb = cp.tile([128, 1], BF16, tag="onesb")
            zblkb = cp.tile([128, 128], BF16, tag="zblkb")
            wotp = pp.tile([128, 4 * D], F32R, tag="wotp")  # wo tiles, concat
            nc.gpsimd.memset(denomA[:], 1.0)
            nc.gpsimd.memset(denomB[:], 1.0)

            vo = [[None] * NST, [None] * NST]
            with tc.tile_pool(name="vop", bufs=1) as vp:  # spans C..D
                with (
                    tc.tile_pool(name="vtbuf", bufs=1) as vtb,
                    tc.tile_pool(name="rotps", bufs=2, space="PSUM") as rpp,
                    tc.tile_pool(name="ropet", bufs=2) as rtp,
                ):
                    vT = vtb.tile([128, S], F32R, tag="vT")

                    def emit_rope(n):
                        ncol = slice(n * 512, (n + 1) * 512)
                        for tl in [*qT, kT]:
                            rps = rpp.tile([128, 512], F32, tag="rps")
                            nc.tensor.matmul(rps[:], protb[:], tl[:, ncol],
                                             start=True, stop=True)
                            tmp = rtp.tile([128, 512], F32, tag="ropetmp")
                            nc.vector.tensor_mul(tmp[:], tl[:, ncol], cosb[:, ncol])
                            nc.vector.tensor_mul(rps[:], rps[:], sinb[:, ncol])
                            nc.vector.tensor_add(tl[:, ncol], tmp[:], rps[:])

                    # ---------- Phase A: qkv^T = W^T @ x^T, + RoPE ----------
                    with (
                        tc.tile_pool(name="wts", bufs=1) as wp,
                        tc.tile_pool(name="xin", bufs=16) as xp,
                        tc.tile_pool(name="qkvps", bufs=6, space="PSUM") as pqkv,
                    ):
                        wqk = [wp.tile([128, 8 * HD], F32R, tag=f"wq{k}", name=f"wqk{k}") for k in range(NKT)]
                        wkk = [wp.tile([128, 2 * HD], F32R, tag=f"wk{k}", name=f"wkk{k}") for k in range(NKT)]
                        wvk = [wp.tile([128, 2 * HD], F32R, tag=f"wv{k}", name=f"wvk{k}") for k in range(NKT)]
                        # DMA order: per-k weights interleaved with chunk-0 x
                        # tiles so the first matmuls can start ~2us in.
                        xk0 = []
                        for k in range(NKT):
                            r = slice(k * 128, (k + 1) * 128)
                            nc.sync.dma_start(wqk[k][:], wq[r, :])
                            nc.sync.dma_start(wkk[k][:], wk[r, :])
                            nc.sync.dma_start(wvk[k][:], wv[r, :])
                            xk = xp.tile([128, 512], F32R, tag="xk", name=f"xk0_{k}")
                            eng = nc.sync if k % 2 == 0 else nc.scalar
                            eng.dma_start(xk[:], xT[r, 0:512])
                            xk0.append(xk)
                        nc.scalar.dma_start(protb[:], prot[:])
                        nc.scalar.dma_start(cosb[:], cosT2[:])
                        nc.scalar.dma_start(sinb[:], sinT2[:])
                        nc.scalar.dma_start(trib[:], tri[:])
                        nc.scalar.dma_start(identb[:], ident[:])
                        nc.scalar.dma_start(selAb[:], selA[:])
                        nc.scalar.dma_start(selBb[:], selB[:])
                        nc.scalar.dma_start(onesb[:], onescol[:])
                        nc.scalar.dma_start(zblkb[:], zblk[:])

                        for n in range(NCHUNK):
                            ncol = slice(n * 512, (n + 1) * 512)
                            accs = [pqkv.tile([128, 512], F32, tag="qkvacc", name=f"acc{n}_{m}") for m in range(6)]
                            for k in range(NKT):
                                if n == 0:
                                    xk = xk0[k]
                                else:
                                    xk = xp.tile([128, 512], F32R, tag="xk")
                                    eng = nc.sync if k % 2 == 0 else nc.scalar
                                    eng.dma_start(xk[:], xT[k * 128:(k + 1) * 128, ncol])
                                st = k == 0
                                sp = k == NKT - 1
                                for t in range(4):
                                    nc.tensor.matmul(
                                        accs[t][:], wqk[k][:, t * 128:(t + 1) * 128],
                                        xk[:], start=st, stop=sp)
                                nc.tensor.matmul(accs[4][:], wkk[k][:], xk[:], start=st, stop=sp)
                                nc.tensor.matmul(accs[5][:], wvk[k][:], xk[:], start=st, stop=sp)
                            # drains alternate DVE / Act so accs free quickly
                            for m, tl in enumerate([*qT, kT]):
                                if m % 2 == 0:
                                    nc.vector.tensor_copy(tl[:, ncol], accs[m][:])
                                else:
                                    nc.scalar.copy(tl[:, ncol], accs[m][:])
                            nc.scalar.copy(vT[:, ncol], accs[5][:])
                            if n == 2:
                                for k4 in range(4):
                                    nc.scalar.dma_start(
                                        wotp[:, k4 * D:(k4 + 1) * D],
                                        wo[k4 * 128:(k4 + 1) * 128, :])
                            # RoPE one chunk behind: its rot matmul depends on
                            # the drain above, so running it inside the next
                            # chunk's matmul stream keeps the PE busy.
                            if n > 0:
                                emit_rope(n - 1)

                    # ---------- Phase C: v natural tiles [128, 65] bf16 ----------
                    with tc.tile_pool(name="vtp", bufs=2, space="PSUM") as vtp:
                        for i in range(NST):
                            for g in range(2):
                                vps = vtp.tile([128, 64], F32R, tag="vps")
                                nc.tensor.transpose(
                                    vps[:], vT[g * 64:(g + 1) * 64, i * 128:(i + 1) * 128],
                                    identb[g * 64:(g + 1) * 64, :])
                                vt = vp.tile([128, 65], BF16, tag=f"vo{g}_{i}", name=f"vo{g}_{i}")
                                nc.vector.tensor_copy(vt[:, 0:64], vps[:])
                                nc.vector.tensor_copy(vt[:, 64:65], onesb[:])
                                vo[g][i] = vt
                        emit_rope(NCHUNK - 1)

                # ---------- Phase D: attention + fused normalize/o_proj ----------
                with (
                    tc.tile_pool(name="esb", bufs=18) as ep,
                    tc.tile_pool(name="rcpp", bufs=2) as rcp_,
                    tc.tile_pool(name="oout", bufs=3) as op,
                    tc.tile_pool(name="sps", bufs=2, space="PSUM") as sp_,
                    tc.tile_pool(name="avp", bufs=2, space="PSUM") as ap_,
                    tc.tile_pool(name="ops", bufs=2, space="PSUM") as opp,
                ):
                    backlog = deque()

                    def drain_backlog(k):
                        while backlog and k:
                            backlog.popleft()()
                            k -= 1

                    for j in range(NCHUNK):
                        jcol = slice(j * 512, (j + 1) * 512)
                        ni = 4 * j + 4
                        nh = ni // 2
                        for t in range(4):
                            for sub in range(2):
                                pb = slice(64 * sub, 64 * sub + 64)
                                g = sub
                                avs = ap_.tile([65, 512], F32, tag="avacc",
                                               name=f"av{t}_{j}_{sub}")
                                esl = []
                                # score/exp chain: two key blocks per PSUM
                                # tile, one wide exp each (Act-paced)
                                for h in range(nh):
                                    ssq = sp_.tile([128, 1024], F32, tag="scps")
                                    for s_ in range(2):
                                        i = 2 * h + s_
                                        c0 = max(0, 128 * (i - 4 * j))
                                        ec0 = c0 if 512 - c0 >= 256 else 256
                                        nc.tensor.matmul(
                                            ssq[:, s_ * 512 + ec0:(s_ + 1) * 512],
                                            kT[pb, i * 128:(i + 1) * 128],
                                            qT[t][pb, j * 512 + ec0:(j + 1) * 512],
                                            start=True, stop=True)
                                    es = ep.tile([128, 1024], BF16, tag="es")
                                    nc.scalar.activation(es[:], ssq[:], EXP, scale=SCALE)
                                    for s_ in range(2):
                                        i = 2 * h + s_
                                        c0 = max(0, 128 * (i - 4 * j))
                                        if i >= 4 * j:
                                            nc.vector.tensor_mul(
                                                es[:, s_ * 512 + c0:s_ * 512 + c0 + 128],
                                                es[:, s_ * 512 + c0:s_ * 512 + c0 + 128],
                                                trib[:])
                                        if c0 == 384:
                                            nc.vector.tensor_copy(
                                                es[:, s_ * 512 + 256:s_ * 512 + 384],
                                                zblkb[:])
                                    esl.append(es)
                                    drain_backlog(1)

                                # AV matmuls: one same-config block, run one
                                # pass behind via the backlog
                                def make_av(t=t, sub=sub, g=g, avs=avs, esl=esl,
                                            jv=j, niv=ni, pbv=pb, jc=jcol):
                                    def av_block(i0, i1):
                                        def emit():
                                            for i in range(i0, i1):
                                                c0 = max(0, 128 * (i - 4 * jv))
                                                av0 = c0 if c0 < 384 else 256
                                                s_ = i % 2
                                                es = esl[i // 2]
                                                nc.tensor.matmul(
                                                    avs[:, av0:512], vo[g][i][:],
                                                    es[:, s_ * 512 + av0:(s_ + 1) * 512],
                                                    start=(i == 0), stop=(i == niv - 1))
                                        return emit

                                    def drain():
                                        nc.vector.tensor_copy(qT[t][pbv, jc], avs[0:64, :])
                                        dst = denomA if sub == 0 else denomB
                                        nc.vector.tensor_copy(
                                            dst[32 * t:32 * t + 1, jc], avs[64:65, :])
                                    items = [av_block(i0, min(i0 + 4, niv))
                                             for i0 in range(0, niv, 4)]
                                    items.append(drain)
                                    return items
                                backlog.extend(make_av())

                        # flush: last pass's AVs must land before normalize
                        drain_backlog(len(backlog))

                        # normalize chunk j across all heads
                        nc.vector.reciprocal_approx_fast(
                            out=denomA[:, jcol], in_=denomA[:, jcol])
                        nc.vector.reciprocal_approx_fast(
                            out=denomB[:, jcol], in_=denomB[:, jcol])
                        rA = rcp_.tile([128, 512], F32R, tag="rA")
                        rB = rcp_.tile([128, 512], F32R, tag="rB")
                        nc.vector.tensor_copy(rA[:], denomA[:, jcol])
                        nc.vector.tensor_copy(rB[:], denomB[:, jcol])
                        for t in range(4):
                            tsl = slice(t * 128, (t + 1) * 128)
                            bpsw = sp_.tile([128, 1024], F32, tag="scps")
                            bps = bpsw[:, 0:512]
                            nc.tensor.matmul(bps, selAb[:, tsl], rA[:],
                                             start=True, stop=False)
                            nc.tensor.matmul(bps, selBb[:, tsl], rB[:],
                                             start=False, stop=True)
                            nc.vector.tensor_mul(qT[t][:, jcol], qT[t][:, jcol], bps)

                        # o_proj rows for this chunk -> backlog (run during
                        # the next chunk's Act-paced score phase)
                        def make_oproj(jv=j):
                            items = []
                            last = jv == NCHUNK - 1
                            for st in range(4 * jv, 4 * jv + 4):
                                stc = slice(st * 128, (st + 1) * 128)
                                for dch in range(NCHUNK):
                                    def emit(st=st, stc=stc, dch=dch, last=last):
                                        ops = opp.tile([128, 512], F32, tag="opps")
                                        for kt in range(4):
                                            nc.tensor.matmul(
                                                ops[:], qT[kt][:, stc],
                                                wotp[:, kt * D + dch * 512:kt * D + (dch + 1) * 512],
                                                start=(kt == 0), stop=(kt == 3))
                                        oo = op.tile([128, 512], F32, tag="oo")
                                        nc.vector.tensor_copy(oo[:], ops[:])
                                        nc.sync.dma_start(
                                            opart[stc, dch * 512:(dch + 1) * 512],
                                            oo[:])
                                    items.append(emit)
                            return items
                        backlog.extend(make_oproj())

                    drain_backlog(len(backlog))

    nc.compile()
    return nc


_PROGRAM = None


def _get_program():
    global _PROGRAM
    if _PROGRAM is None:
        _PROGRAM = _build_program()
    return _PROGRAM


def _make_in_maps(x, cos, sin, Wq, Wk, Wv, Wo):
    cosT = np.ascontiguousarray(cos.T.astype(np.float32))      # [64, S]
    sinT = np.ascontiguousarray(sin.T.astype(np.float32))
    cosT2 = np.tile(cosT, (2, 1))
    sinT2 = np.tile(sinT, (2, 1))
    tri = (np.arange(128)[None, :] >= np.arange(128)[:, None])
    tri = tri.astype(np.float32).astype(np.dtype("bfloat16") if hasattr(np, "bfloat16") else np.float32)
    import ml_dtypes
    tri = (np.arange(128)[None, :] >= np.arange(128)[:, None]).astype(ml_dtypes.bfloat16)
    ident = np.tile(np.eye(64, dtype=np.float32), (2, 1))
    # rotate-half permutation (sign folded): rot[m] = -tl[m+32] (m%64<32),
    # rot[m] = +tl[m-32] (m%64>=32); out[m,n] = sum_k prot[k,m]*tl[k,n]
    prot = np.zeros((128, 128), dtype=np.float32)
    for m in range(128):
        if m % 64 < 32:
            prot[m + 32, m] = -1.0
        else:
            prot[m - 32, m] = 1.0
    selA = np.zeros((128, 512), dtype=np.float32)
    selB = np.zeros((128, 512), dtype=np.float32)
    for t in range(4):
        selA[32 * t, 128 * t:128 * t + 64] = 1.0
        selB[32 * t, 128 * t + 64:128 * t + 128] = 1.0

    perm = [0, 4, 1, 5, 2, 6, 3, 7]
    in_maps = []
    for c in range(8):
        b, q = c // 4, c % 4
        idx = np.concatenate([np.arange(HD) + (8 * q + j) * HD for j in perm])
        in_maps.append({
            "xT": np.ascontiguousarray(x[b].T.astype(np.float32)),
            "wq": np.ascontiguousarray(Wq[:, idx].astype(np.float32)),
            "wk": np.ascontiguousarray(Wk[:, 2 * q * HD:(2 * q + 2) * HD].astype(np.float32)),
            "wv": np.ascontiguousarray(Wv[:, 2 * q * HD:(2 * q + 2) * HD].astype(np.float32)),
            "wo": np.ascontiguousarray(Wo[idx, :].astype(np.float32)),
            "cosT2": cosT2,
            "sinT2": sinT2,
            "prot": prot,
            "tri": tri,
            "ident": ident,
            "selA": selA,
            "selB": selB,
            "onescol": np.ones((128, 1), dtype=ml_dtypes.bfloat16),
            "zblk": np.zeros((128, 128), dtype=ml_dtypes.bfloat16),
        })
    return in_maps


def _execute(in_maps, trace=False):
    nc = _get_program()
    return bass_utils.run_bass_kernel_spmd(
        nc, in_maps, core_ids=list(range(8)), trace=trace)


def kernel(x, cos, sin, Wq, Wk, Wv, Wo):
    in_maps = _make_in_maps(x, cos, sin, Wq, Wk, Wv, Wo)
    res = _execute(in_maps, trace=False)
    parts = [r["opart"] for r in res.results]
    out = np.empty((B, S, D), dtype=np.float32)
    for b in range(B):
        p = parts[4 * b:4 * b + 4]
        out[b] = (p[0] + p[1]) + (p[2] + p[3])
    return out


# revision 14
# speedup vs baseline: 1.8245x; 1.0255x over previous
"""GQA (RoPE + causal softmax) Trainium2 Bass kernel, 8-core SPMD.

Sharding: DP over batch (2) x TP over KV groups (4 quarters of heads).
Core c handles batch c//4 and head quarter c%4 (8 q-heads, 2 kv-heads).
Each core computes a partial o_proj ([S, D]); host sums 4 partials per batch.

Matmuls run in float32r (1 col/cyc at N>=256, PE @2.4GHz); attention
probabilities and V are bf16. Everything on-chip is in "transposed"
layout (feature dim on partitions), so x^T is the only host-side prep.

Schedule (PE and Act kept concurrently busy):
 - Phase A: qkv^T = W^T @ x^T per 512-col chunk. Chunk-0 x DMAs are
   interleaved with the weight DMAs so the first matmul starts ~2us in.
   RoPE for chunk n (PE permutation matmul + DVE mul/mul/add) is
   emitted after chunk n+1's projection matmuls so it never stalls the
   PE. PSUM drains alternate DVE/Act.
 - Phase C: v natural tiles [128, 65] bf16 (ones column -> denominator
   rides the AV matmul).
 - Phase D (j-outer): scores for two key blocks go into one [128,1024]
   PSUM tile and get a single wide exp into a persistent bf16 SBUF
   tile. The score->exp chain is Act-paced; the PE's AV matmuls (one
   same-config block per head-pass, run one pass behind) and o_proj
   tiles (one chunk behind) are drawn from a backlog to fill the PE
   during Act waits. Chunk normalization uses reciprocal_approx_fast
   and a select-matmul partition broadcast; attention output reuses
   the qT tiles (their columns are dead after the chunk's scores).
"""

from collections import deque

import numpy as np

import concourse.bass as bass
import concourse.mybir as mybir
import concourse.tile as tile
from concourse import bacc, bass_utils

B, S, D = 2, 2048, 2048
H, KV, HD = 32, 8, 64
REP = H // KV
SCALE = 1.0 / 8.0  # 1/sqrt(HD)

F32 = mybir.dt.float32
F32R = mybir.dt.float32r
BF16 = mybir.dt.bfloat16
EXP = mybir.ActivationFunctionType.Exp

NCHUNK = S // 512        # 4 sq chunks of 512
NKT = D // 128           # 16 k-tiles over D
NST = S // 128           # 16 sk/s tiles


def _build_program():
    nc = bacc.Bacc()

    xT = nc.dram_tensor("xT", [D, S], F32R, kind="ExternalInput").ap()
    wq = nc.dram_tensor("wq", [D, 8 * HD], F32R, kind="ExternalInput").ap()
    wk = nc.dram_tensor("wk", [D, 2 * HD], F32R, kind="ExternalInput").ap()
    wv = nc.dram_tensor("wv", [D, 2 * HD], F32R, kind="ExternalInput").ap()
    wo = nc.dram_tensor("wo", [8 * HD, D], F32R, kind="ExternalInput").ap()
    cosT2 = nc.dram_tensor("cosT2", [128, S], F32, kind="ExternalInput").ap()
    sinT2 = nc.dram_tensor("sinT2", [128, S], F32, kind="ExternalInput").ap()
    prot = nc.dram_tensor("prot", [128, 128], F32R, kind="ExternalInput").ap()
    tri = nc.dram_tensor("tri", [128, 128], BF16, kind="ExternalInput").ap()
    ident = nc.dram_tensor("ident", [128, 64], F32R, kind="ExternalInput").ap()
    selA = nc.dram_tensor("selA", [128, 512], F32R, kind="ExternalInput").ap()
    selB = nc.dram_tensor("selB", [128, 512], F32R, kind="ExternalInput").ap()
    onescol = nc.dram_tensor("onescol", [128, 1], BF16, kind="ExternalInput").ap()
    zblk = nc.dram_tensor("zblk", [128, 128], BF16, kind="ExternalInput").ap()
    opart = nc.dram_tensor("opart", [S, D], F32, kind="ExternalOutput").ap()

    with tile.TileContext(nc) as tc:
        with (
            tc.tile_pool(name="persist", bufs=1) as pp,
            tc.tile_pool(name="consts", bufs=1) as cp,
        ):
            # persistent SBUF: q^T/k^T (qT doubles as attn-out storage),
            # denominators, small constants
            qT = [pp.tile([128, S], F32R, tag=f"qT{t}", name=f"qT{t}") for t in range(4)]
            kT = pp.tile([128, S], F32R, tag="kT")
            denomA = pp.tile([128, S], F32, tag="denomA")
            denomB = pp.tile([128, S], F32, tag="denomB")
            cosb = cp.tile([128, S], F32, tag="cosb")
            sinb = cp.tile([128, S], F32, tag="sinb")
            protb = cp.tile([128, 128], F32R, tag="protb")
            trib = cp.tile([128, 128], BF16, tag="trib")
            identb = cp.tile([128, 64], F32R, tag="identb")
            selAb = cp.tile([128, 512], F32R, tag="selAb")
            selBb = cp.tile([128, 512], F32R, tag="selBb")
            onesb = cp.tile([128, 1], BF16, tag="onesb")
            zblkb = cp.tile([128, 128], BF16, tag="zblkb")
            wotp = pp.tile([128, 4 * D], F32R, tag="wotp")  # wo tiles, concat
            nc.gpsimd.memset(denomA[:], 1.0)
            nc.gpsimd.memset(denomB[:], 1.0)

            vo = [[None] * NST, [None] * NST]
            with tc.tile_pool(name="vop", bufs=1) as vp:  # spans C..D
                with (
                    tc.tile_pool(name="vtbuf", bufs=1) as vtb,
                    tc.tile_pool(name="rotps", bufs=2, space="PSUM") as rpp,
                    tc.tile_pool(name="ropet", bufs=2) as rtp,
                ):
                    vT = vtb.tile([128, S], F32R, tag="vT")

                    def emit_rope(n):
                        ncol = slice(n * 512, (n + 1) * 512)
                        for tl in [*qT, kT]:
                            rps = rpp.tile([128, 512], F32, tag="rps")
                            nc.tensor.matmul(rps[:], protb[:], tl[:, ncol],
                                             start=True, stop=True)
                            tmp = rtp.tile([128, 512], F32, tag="ropetmp")
                            nc.vector.tensor_mul(tmp[:], tl[:, ncol], cosb[:, ncol])
                            nc.vector.tensor_mul(rps[:], rps[:], sinb[:, ncol])
                            nc.vector.tensor_add(tl[:, ncol], tmp[:], rps[:])

                    # ---------- Phase A: qkv^T = W^T @ x^T, + RoPE ----------
                    with (
                        tc.tile_pool(name="wts", bufs=1) as wp,
                        tc.tile_pool(name="xin", bufs=16) as xp,
                        tc.tile_pool(name="qkvps", bufs=6, space="PSUM") as pqkv,
                    ):
                        wqk = [wp.tile([128, 8 * HD], F32R, tag=f"wq{k}", name=f"wqk{k}") for k in range(NKT)]
                        wkk = [wp.tile([128, 2 * HD], F32R, tag=f"wk{k}", name=f"wkk{k}") for k in range(NKT)]
                        wvk = [wp.tile([128, 2 * HD], F32R, tag=f"wv{k}", name=f"wvk{k}") for k in range(NKT)]
                        # DMA order: per-k weights interleaved with chunk-0 x
                        # tiles so the first matmuls can start ~2us in.
                        xk0 = []
                        for k in range(NKT):
                            r = slice(k * 128, (k + 1) * 128)
                            nc.sync.dma_start(wqk[k][:], wq[r, :])
                            nc.sync.dma_start(wkk[k][:], wk[r, :])
                            nc.sync.dma_start(wvk[k][:], wv[r, :])
                            xk = xp.tile([128, 512], F32R, tag="xk", name=f"xk0_{k}")
                            eng = nc.sync if k % 2 == 0 else nc.scalar
                            eng.dma_start(xk[:], xT[r, 0:512])
                            xk0.append(xk)
                        nc.scalar.dma_start(protb[:], prot[:])
                        nc.scalar.dma_start(cosb[:], cosT2[:])
                        nc.scalar.dma_start(sinb[:], sinT2[:])
                        nc.scalar.dma_start(trib[:], tri[:])
                        nc.scalar.dma_start(identb[:], ident[:])
                        nc.scalar.dma_start(selAb[:], selA[:])
                        nc.scalar.dma_start(selBb[:], selB[:])
                        nc.scalar.dma_start(onesb[:], onescol[:])
                        nc.scalar.dma_start(zblkb[:], zblk[:])

                        for n in range(NCHUNK):
                            ncol = slice(n * 512, (n + 1) * 512)
                            accs = [pqkv.tile([128, 512], F32, tag="qkvacc", name=f"acc{n}_{m}") for m in range(6)]
                            for k in range(NKT):
                                if n == 0:
                                    xk = xk0[k]
                                else:
                                    xk = xp.tile([128, 512], F32R, tag="xk")
                                    eng = nc.sync if k % 2 == 0 else nc.scalar
                                    eng.dma_start(xk[:], xT[k * 128:(k + 1) * 128, ncol])
                                st = k == 0
                                sp = k == NKT - 1
                                for t in range(4):
                                    nc.tensor.matmul(
                                        accs[t][:], wqk[k][:, t * 128:(t + 1) * 128],
                                        xk[:], start=st, stop=sp)
                                nc.tensor.matmul(accs[4][:], wkk[k][:], xk[:], start=st, stop=sp)
                                nc.tensor.matmul(accs[5][:], wvk[k][:], xk[:], start=st, stop=sp)
                            # drains alternate DVE / Act so accs free quickly
                            for m, tl in enumerate([*qT, kT]):
                                if m % 2 == 0:
                                    nc.vector.tensor_copy(tl[:, ncol], accs[m][:])
                                else:
                                    nc.scalar.copy(tl[:, ncol], accs[m][:])
                            nc.scalar.copy(vT[:, ncol], accs[5][:])
                            if n == 2:
                                for k4 in range(4):
                                    nc.scalar.dma_start(
                                        wotp[:, k4 * D:(k4 + 1) * D],
                                        wo[k4 * 128:(k4 + 1) * 128, :])
                            # RoPE one chunk behind: its rot matmul depends on
                            # the drain above, so running it inside the next
                            # chunk's matmul stream keeps the PE busy.
                            if n > 0:
                                emit_rope(n - 1)

                    # ---------- Phase C: v natural tiles [128, 65] bf16 ----------
                    with tc.tile_pool(name="vtp", bufs=2, space="PSUM") as vtp:
                        for i in range(NST):
                            for g in range(2):
                                vps = vtp.tile([128, 64], F32R, tag="vps")
                                nc.tensor.transpose(
                                    vps[:], vT[g * 64:(g + 1) * 64, i * 128:(i + 1) * 128],
                                    identb[g * 64:(g + 1) * 64, :])
                                vt = vp.tile([128, 65], BF16, tag=f"vo{g}_{i}", name=f"vo{g}_{i}")
                                nc.vector.tensor_copy(vt[:, 0:64], vps[:])
                                nc.vector.tensor_copy(vt[:, 64:65], onesb[:])
                                vo[g][i] = vt
                        emit_rope(NCHUNK - 1)

                # ---------- Phase D: attention + fused normalize/o_proj ----------
                with (
                    tc.tile_pool(name="esb", bufs=18) as ep,
                    tc.tile_pool(name="rcpp", bufs=2) as rcp_,
                    tc.tile_pool(name="oout", bufs=3) as op,
                    tc.tile_pool(name="sps", bufs=2, space="PSUM") as sp_,
                    tc.tile_pool(name="avp", bufs=2, space="PSUM") as ap_,
                    tc.tile_pool(name="ops", bufs=2, space="PSUM") as opp,
                ):
                    backlog = deque()

                    def drain_backlog(k):
                        while backlog and k:
                            backlog.popleft()()
                            k -= 1

                    for j in range(NCHUNK):
                        jcol = slice(j * 512, (j + 1) * 512)
                        ni = 4 * j + 4
                        nh = ni // 2
                        for t in range(4):
                            for sub in range(2):
                                pb = slice(64 * sub, 64 * sub + 64)
                                g = sub
                                avs = ap_.tile([65, 512], F32, tag="avacc",
                                               name=f"av{t}_{j}_{sub}")
                                esl = []
                                # score/exp chain: two key blocks per PSUM
                                # tile, one wide exp each (Act-paced)
                                for h in range(nh):
                                    ssq = sp_.tile([128, 1024], F32, tag="scps")
                                    for s_ in range(2):
                                        i = 2 * h + s_
                                        c0 = max(0, 128 * (i - 4 * j))
                                        ec0 = c0 if 512 - c0 >= 256 else 256
                                        nc.tensor.matmul(
                                            ssq[:, s_ * 512 + ec0:(s_ + 1) * 512],
                                            kT[pb, i * 128:(i + 1) * 128],
                                            qT[t][pb, j * 512 + ec0:(j + 1) * 512],
                                            start=True, stop=True)
                                    es = ep.tile([128, 1024], BF16, tag="es")
                                    nc.scalar.activation(es[:], ssq[:], EXP, scale=SCALE)
                                    for s_ in range(2):
                                        i = 2 * h + s_
                                        c0 = max(0, 128 * (i - 4 * j))
                                        if i >= 4 * j:
                                            nc.vector.tensor_mul(
                                                es[:, s_ * 512 + c0:s_ * 512 + c0 + 128],
                                                es[:, s_ * 512 + c0:s_ * 512 + c0 + 128],
                                                trib[:])
                                        if c0 == 384:
                                            nc.vector.tensor_copy(
                                                es[:, s_ * 512 + 256:s_ * 512 + 384],
                                                zblkb[:])
                                    esl.append(es)
                                    if h % 2 == 1:
                                        drain_backlog(2)

                                # AV matmuls: one same-config block, run one
                                # pass behind via the backlog
                                def make_av(t=t, sub=sub, g=g, avs=avs, esl=esl,
                                            jv=j, niv=ni, pbv=pb, jc=jcol):
                                    def av_block(i0, i1):
                                        def emit():
                                            for i in range(i0, i1):
                                                c0 = max(0, 128 * (i - 4 * jv))
                                                av0 = c0 if c0 < 384 else 256
                                                s_ = i % 2
                                                es = esl[i // 2]
                                                nc.tensor.matmul(
                                                    avs[:, av0:512], vo[g][i][:],
                                                    es[:, s_ * 512 + av0:(s_ + 1) * 512],
                                                    start=(i == 0), stop=(i == niv - 1))
                                        return emit

                                    def drain():
                                        nc.vector.tensor_copy(qT[t][pbv, jc], avs[0:64, :])
                                        dst = denomA if sub == 0 else denomB
                                        nc.vector.tensor_copy(
                                            dst[32 * t:32 * t + 1, jc], avs[64:65, :])
                                    items = [av_block(i0, min(i0 + 4, niv))
                                             for i0 in range(0, niv, 4)]
                                    items.append(drain)
                                    return items
                                backlog.extend(make_av())

                        # flush: last pass's AVs must land before normalize
                        drain_backlog(len(backlog))

                        # normalize chunk j across all heads
                        nc.vector.reciprocal_approx_fast(
                            out=denomA[:, jcol], in_=denomA[:, jcol])
                        nc.vector.reciprocal_approx_fast(
                            out=denomB[:, jcol], in_=denomB[:, jcol])
                        rA = rcp_.tile([128, 512], F32R, tag="rA")
                        rB = rcp_.tile([128, 512], F32R, tag="rB")
                        nc.vector.tensor_copy(rA[:], denomA[:, jcol])
                        nc.vector.tensor_copy(rB[:], denomB[:, jcol])
                        for t in range(4):
                            tsl = slice(t * 128, (t + 1) * 128)
                            bpsw = sp_.tile([128, 1024], F32, tag="scps")
                            bps = bpsw[:, 0:512]
                            nc.tensor.matmul(bps, selAb[:, tsl], rA[:],
                                             start=True, stop=False)
                            nc.tensor.matmul(bps, selBb[:, tsl], rB[:],
                                             start=False, stop=True)
                            nc.vector.tensor_mul(qT[t][:, jcol], qT[t][:, jcol], bps)

                        # o_proj rows for this chunk -> backlog (run during
                        # the next chunk's Act-paced score phase)
                        def make_oproj(jv=j):
                            items = []
                            last = jv == NCHUNK - 1
                            for st in range(4 * jv, 4 * jv + 4):
                                stc = slice(st * 128, (st + 1) * 128)
                                for dch in range(NCHUNK):
                                    def emit(st=st, stc=stc, dch=dch, last=last):
                                        ops = opp.tile([128, 512], F32, tag="opps")
                                        for kt in range(4):
                                            nc.tensor.matmul(
                                                ops[:], qT[kt][:, stc],
                                                wotp[:, kt * D + dch * 512:kt * D + (dch + 1) * 512],
                                                start=(kt == 0), stop=(kt == 3))
                                        oo = op.tile([128, 512], F32, tag="oo")
                                        nc.vector.tensor_copy(oo[:], ops[:])
                                        nc.sync.dma_start(
                                            opart[stc, dch * 512:(dch + 1) * 512],
                                            oo[:])
                                    items.append(emit)
                            return items
                        backlog.extend(make_oproj())

                    drain_backlog(len(backlog))

    nc.compile()
    return nc


_PROGRAM = None


def _get_program():
    global _PROGRAM
    if _PROGRAM is None:
        _PROGRAM = _build_program()
    return _PROGRAM


def _make_in_maps(x, cos, sin, Wq, Wk, Wv, Wo):
    cosT = np.ascontiguousarray(cos.T.astype(np.float32))      # [64, S]
    sinT = np.ascontiguousarray(sin.T.astype(np.float32))
    cosT2 = np.tile(cosT, (2, 1))
    sinT2 = np.tile(sinT, (2, 1))
    tri = (np.arange(128)[None, :] >= np.arange(128)[:, None])
    tri = tri.astype(np.float32).astype(np.dtype("bfloat16") if hasattr(np, "bfloat16") else np.float32)
    import ml_dtypes
    tri = (np.arange(128)[None, :] >= np.arange(128)[:, None]).astype(ml_dtypes.bfloat16)
    ident = np.tile(np.eye(64, dtype=np.float32), (2, 1))
    # rotate-half permutation (sign folded): rot[m] = -tl[m+32] (m%64<32),
    # rot[m] = +tl[m-32] (m%64>=32); out[m,n] = sum_k prot[k,m]*tl[k,n]
    prot = np.zeros((128, 128), dtype=np.float32)
    for m in range(128):
        if m % 64 < 32:
            prot[m + 32, m] = -1.0
        else:
            prot[m - 32, m] = 1.0
    selA = np.zeros((128, 512), dtype=np.float32)
    selB = np.zeros((128, 512), dtype=np.float32)
    for t in range(4):
        selA[32 * t, 128 * t:128 * t + 64] = 1.0
        selB[32 * t, 128 * t + 64:128 * t + 128] = 1.0

    perm = [0, 4, 1, 5, 2, 6, 3, 7]
    in_maps = []
    for c in range(8):
        b, q = c // 4, c % 4
        idx = np.concatenate([np.arange(HD) + (8 * q + j) * HD for j in perm])
        in_maps.append({
            "xT": np.ascontiguousarray(x[b].T.astype(np.float32)),
            "wq": np.ascontiguousarray(Wq[:, idx].astype(np.float32)),
            "wk": np.ascontiguousarray(Wk[:, 2 * q * HD:(2 * q + 2) * HD].astype(np.float32)),
            "wv": np.ascontiguousarray(Wv[:, 2 * q * HD:(2 * q + 2) * HD].astype(np.float32)),
            "wo": np.ascontiguousarray(Wo[idx, :].astype(np.float32)),
            "cosT2": cosT2,
            "sinT2": sinT2,
            "prot": prot,
            "tri": tri,
            "ident": ident,
            "selA": selA,
            "selB": selB,
            "onescol": np.ones((128, 1), dtype=ml_dtypes.bfloat16),
            "zblk": np.zeros((128, 128), dtype=ml_dtypes.bfloat16),
        })
    return in_maps


def _execute(in_maps, trace=False):
    nc = _get_program()
    return bass_utils.run_bass_kernel_spmd(
        nc, in_maps, core_ids=list(range(8)), trace=trace)


def kernel(x, cos, sin, Wq, Wk, Wv, Wo):
    in_maps = _make_in_maps(x, cos, sin, Wq, Wk, Wv, Wo)
    res = _execute(in_maps, trace=False)
    parts = [r["opart"] for r in res.results]
    out = np.empty((B, S, D), dtype=np.float32)
    for b in range(B):
        p = parts[4 * b:4 * b + 4]
        out[b] = (p[0] + p[1]) + (p[2] + p[3])
    return out
